# revision 15
# baseline (speedup 1.0000x reference)
"""CityExpertMoE Trainium2 kernel — mixed fp8/bf16 expert-parallel design.

Phase 1 (data-parallel over tokens): per core, upload x^T for its 1024
tokens once; the PE computes A = x @ [gate*gamma | 1] (router logits
numerator + column sums) and s2 = colsum(x^2) with x-slabs stationary.
Host derives LayerNorm mu/rstd, exact softmax/top-2 routing (f64), and
combine weights.

Host dispatch: per expert, tokens sorted ascending by combine weight cw;
the lowest-cw slots (including zero padding) go to an fp8 section, the
highest-cw tokens to a bf16 section. Quantization error enters the
output scaled by cw, so fp8 e4m3 (DoubleRow, ~1.44x tensor rate) on
low-cw slots keeps total L2 error ~1.6e-2 while accelerating ~71% of
the FLOPs.

Phase 2 (expert-parallel): core e runs expert e's FFN. bf16 section
first (weights resident), then fp8 section whose weights reuse the
bf16 weight SBUF slots (tag ring, WAR-tracked). Host combine:
scatter-add + residual (+ cw*b2 when b2 nonzero).
"""

import sys
import types

import numpy as np
import ml_dtypes

# If BASS_TRACE is set but the axon NTFF hook shim is absent, bass_utils
# would fail importing antenv.axon_hooks; register a no-op fallback.
try:
    import antenv.axon_hooks  # noqa: F401
except ImportError:
    _m = types.ModuleType("antenv.axon_hooks")
    _m._hook = None
    _m.set_axon_ntff_profile_hook = lambda h: setattr(_m, "_hook", h)
    _m.get_axon_ntff_profile_hook = lambda: _m._hook
    sys.modules["antenv.axon_hooks"] = _m
    try:
        import antenv
        antenv.axon_hooks = _m
    except ImportError:
        pass

import concourse.bass as bass
import concourse.mybir as mybir
import concourse.tile as tile
from concourse import bacc
from concourse.bass_utils import run_bass_kernel_spmd

F32 = mybir.dt.float32
F32R = mybir.dt.float32r
BF16 = mybir.dt.bfloat16
F8 = mybir.dt.float8e4
AF = mybir.ActivationFunctionType
ALU = mybir.AluOpType
DR = mybir.MatmulPerfMode.DoubleRow

E4NP = ml_dtypes.float8_e4m3
BFNP = ml_dtypes.bfloat16

B, L, D, H, E, TOP_K = 4, 2048, 1024, 4096, 8, 2
T = B * L               # 8192 tokens total
N_CORES = 8
TC = T // N_CORES       # 1024 tokens per core in phase 1
KT = D // 128           # 8 k-tiles over D
HT = H // 128           # 32 k-tiles over H
LN_EPS = 1e-5
W1_SCALE = 16.0         # w1 pre-scale before e4m3 cast (std -> ~0.5)
W2_SCALE = 32.0         # w2 pre-scale before e4m3 cast
FP8_FRAC = 0.76471      # fraction of slots (lowest cw) in the fp8 section
H_CENTER = 0.283        # E[gelu(z)], z~N(0,1): subtracted before the fp8
                        # cast of h to cut quantization error; the host adds
                        # back cw*(H_CENTER/W2_SCALE)*colsum(w2q) per expert

_cache: dict = {}
LAST_RESULTS: dict = {}


# ---------------------------------------------------------------- phase 1
def build_phase1():
    """Router/LN statistics: A = x @ [g|1|1] and s2 = colsum(x^2), bf16.

    x^T [D, TC] uploads once in bf16 (2MB): bf16 stationaries get the
    fast weight-load path, unlike fp32/f32r whose weight port runs 4x
    slower. The resulting ~0.6% logit noise is repaired on the host by
    an exact recheck of borderline tokens plus exact top-2 pair logits.
    Output stats [TC, 12] = [A(8) | s1 | s1 | s2 | s2]."""
    nc = bacc.Bacc("TRN2", target_bir_lowering=False, debug=False,
                   num_devices=N_CORES)
    xT_d = nc.dram_tensor("xT", [D, TC], BF16, kind="ExternalInput").ap()
    # 10 columns: 8 gate + 2 ones (even sizes keep every matmul ISA-legal)
    gate9_d = nc.dram_tensor("gate9", [128, KT, 10], BF16,
                             kind="ExternalInput").ap()
    stats_o = nc.dram_tensor("stats", [TC, 12], F32, kind="ExternalOutput").ap()
    NTT = TC // 128

    with tile.TileContext(nc) as tc:
        import contextlib
        with contextlib.ExitStack() as ctx:
            const = ctx.enter_context(tc.tile_pool(name="const", bufs=1))
            big = ctx.enter_context(tc.tile_pool(name="big", bufs=1))
            outp = ctx.enter_context(tc.tile_pool(name="outp", bufs=1))
            psp = ctx.enter_context(
                tc.tile_pool(name="psp", bufs=4, space="PSUM"))

            gate9 = const.tile([128, KT, 10], BF16)
            nc.sync.dma_start(gate9[:], gate9_d[:])

            xT_sb = big.tile([128, KT, TC], BF16)
            sq = big.tile([128, KT, TC], BF16)
            xT_r = xT_d.rearrange("(k p) t -> p k t", p=128)
            for ch in range(2):
                csl = bass.ts(ch, TC // 2)
                nc.sync.dma_start(xT_sb[:, :, csl], xT_r[:, :, csl])
                nc.vector.tensor_mul(sq[:, :, csl], xT_sb[:, :, csl],
                                     xT_sb[:, :, csl])

            stats = outp.tile([128, NTT, 12], F32)
            for t in range(NTT):
                tsl = bass.ts(t, 128)
                pA = psp.tile([128, 10], F32, tag="pA", name=f"pA_{t}")
                for k in range(KT):
                    nc.tensor.matmul(pA[:], xT_sb[:, k, tsl], gate9[:, k, :],
                                     start=(k == 0), stop=(k == KT - 1))
                nc.vector.tensor_copy(stats[:, t, 0:10], pA[:])
                pS = psp.tile([128, 2], F32, tag="pS", name=f"pS_{t}")
                for k in range(KT):
                    # gate9 cols 8:10 are all-ones summing vectors
                    nc.tensor.matmul(pS[:], sq[:, k, tsl], gate9[:, k, 8:10],
                                     start=(k == 0), stop=(k == KT - 1))
                nc.vector.tensor_copy(stats[:, t, 10:12], pS[:])
            nc.sync.dma_start(stats_o.rearrange("(t p) n -> p t n", p=128),
                              stats[:])

    nc.compile()
    return nc


def _blocks_bf16(n):
    """512-token blocks; small tails rebalanced like the baseline."""
    blocks = [512] * (n // 512)
    r = n % 512
    if r:
        if r < 256 and blocks:
            blocks.pop()
            total = 512 + r
            first = ((total + 1) // 2 + 127) // 128 * 128
            blocks.extend([first, total - first])
        else:
            blocks.append(r)
    return blocks


def _blocks_fp8(n):
    blocks = [512] * (n // 512)
    r = n % 512
    if r:
        blocks.append(r)
    return blocks


# ---------------------------------------------------------------- phase 2
def build_phase2(C: int, S: int, act=AF.Gelu):
    """Expert FFN on C slots: CB=C-S bf16 tokens then S fp8 tokens.

    y rows: [0, CB) = bf16 section slots, [CB, C) = fp8 section slots.
    fp8 weights are pre-scaled on host (W1_SCALE/W2_SCALE); the gelu
    activation un-scales mm1 (scale=1/W1_SCALE) and cw8r carries
    cw/W2_SCALE so mm2's scale folds into the existing combine mult."""
    CB = C - S
    bblocks = _blocks_bf16(CB)
    fblocks = _blocks_fp8(S)
    nc = bacc.Bacc("TRN2", target_bir_lowering=False, debug=False,
                   num_devices=N_CORES)
    xbT = nc.dram_tensor("xbT", [D, max(CB, 1)], BF16,
                         kind="ExternalInput").ap()
    xqT = nc.dram_tensor("xqT", [D, max(S, 1)], F8, kind="ExternalInput").ap()
    w1b_d = nc.dram_tensor("w1b", [D, H], BF16, kind="ExternalInput").ap()
    w2b_d = nc.dram_tensor("w2b", [H, D], BF16, kind="ExternalInput").ap()
    w1q_d = nc.dram_tensor("w1q", [D, H], F8, kind="ExternalInput").ap()
    w2q_d = nc.dram_tensor("w2q", [H, D], F8, kind="ExternalInput").ap()
    b1r_d = nc.dram_tensor("b1r", [128, HT], F32, kind="ExternalInput").ap()
    cwbr_d = nc.dram_tensor("cwbr", [128, max(CB // 128, 1)], F32,
                            kind="ExternalInput").ap()
    cw8r_d = nc.dram_tensor("cw8r", [128, max(S // 128, 1)], F32,
                            kind="ExternalInput").ap()
    y_o = nc.dram_tensor("y", [C, D], F32, kind="ExternalOutput").ap()

    xbT_r = xbT.rearrange("(k p) t -> p k t", p=128)
    xqT_r = xqT.rearrange("(k p) t -> p k t", p=128)
    w1b_r = w1b_d.rearrange("(k p) h -> p k h", p=128)
    w2b_r = w2b_d.rearrange("(k p) d -> p k d", p=128)
    w1q_r = w1q_d.rearrange("(k p) h -> p k h", p=128)
    w2q_r = w2q_d.rearrange("(k p) d -> p k d", p=128)

    with tile.TileContext(nc) as tc:
        import contextlib
        with contextlib.ExitStack() as ctx:
            const = ctx.enter_context(tc.tile_pool(name="const", bufs=1))
            wpool = ctx.enter_context(tc.tile_pool(name="w", bufs=2))
            xpool = ctx.enter_context(tc.tile_pool(name="xp", bufs=2))
            hpool = ctx.enter_context(tc.tile_pool(name="h", bufs=34))
            opool = ctx.enter_context(tc.tile_pool(name="o", bufs=2))
            hfpool = ctx.enter_context(tc.tile_pool(name="hf", bufs=2))
            ps1p = ctx.enter_context(
                tc.tile_pool(name="ps1", bufs=4, space="PSUM"))
            ps2p = ctx.enter_context(
                tc.tile_pool(name="ps2", bufs=4, space="PSUM"))

            # ---- DMA preamble: bf16 weights + block-0 activations first;
            # same issue-order trick as the baseline (block-0 x, then w1 in
            # chunks so mm1 can start early, then the rest).
            if CB:
                xb0 = xpool.tile([128, KT, bblocks[0]], BF16, tag="xb",
                                 name="xb_0")
                nc.sync.dma_start(xb0[:], xbT_r[:, :, 0:bblocks[0]])
            w1b_sb = wpool.tile([128, KT, H], BF16, tag="w", name="w1b_sb")
            w2b_sb = wpool.tile([128, HT, D], BF16, tag="w", name="w2b_sb")
            if CB:
                nc.sync.dma_start(w1b_sb[:, :, 0:H // 16],
                                  w1b_r[:, :, 0:H // 16])
            b1_sb = const.tile([128, HT], F32)
            nc.sync.dma_start(b1_sb[:], b1r_d[:])
            cwb_sb = const.tile([128, max(CB // 128, 1)], F32)
            nc.sync.dma_start(cwb_sb[:], cwbr_d[:])
            cw8_sb = const.tile([128, max(S // 128, 1)], F32)
            nc.sync.dma_start(cw8_sb[:], cw8r_d[:])
            if CB:
                nc.sync.dma_start(w1b_sb[:, :, H // 16:H // 8],
                                  w1b_r[:, :, H // 16:H // 8])
                for q in range(1, 8):
                    qsl = bass.ts(q, H // 8)
                    nc.sync.dma_start(w1b_sb[:, :, qsl], w1b_r[:, :, qsl])
                nc.sync.dma_start(w2b_sb[:, 0:HT // 2, :],
                                  w2b_r[:, 0:HT // 2, :])
                nc.sync.dma_start(w2b_sb[:, HT // 2:HT, :],
                                  w2b_r[:, HT // 2:HT, :])
            # fp8 activations are small (~1.25MB); queue them early so the
            # fp8 section never waits on them.
            fxs = []
            tok0 = 0
            for b, blk in enumerate(fblocks):
                xq = xpool.tile([128, KT, blk], F8, tag="xq", name=f"xq_{b}")
                nc.sync.dma_start(xq[:], xqT_r[:, :, tok0:tok0 + blk])
                fxs.append(xq)
                tok0 += blk

            # ---- bf16 section (baseline structure) ----
            tok0 = 0
            for b, blk in enumerate(bblocks):
                if b == 0:
                    xb = xb0
                else:
                    xb = xpool.tile([128, KT, blk], BF16, tag="xb",
                                    name=f"xb_{b}")
                    nc.sync.dma_start(xb[:], xbT_r[:, :, tok0:tok0 + blk])
                hts = []
                for ht in range(HT):
                    ps = ps1p.tile([128, blk], F32, tag="ps1",
                                   name=f"bps1_{b}_{ht}")
                    for k in range(KT):
                        nc.tensor.matmul(
                            ps[:], w1b_sb[:, k, ht * 128:(ht + 1) * 128],
                            xb[:, k, :], start=(k == 0), stop=(k == KT - 1))
                    htile = hpool.tile([128, blk], BF16, tag="h",
                                       name=f"bht_{b}_{ht}")
                    nc.scalar.activation(htile[:], ps[:], act,
                                         bias=b1_sb[:, ht:ht + 1])
                    hts.append(htile)
                # issue the fp8-weight DMAs right after the LAST bf16 mm1:
                # their SBUF slots (w-tag ring) free exactly then, and all
                # earlier y-outs are already queued ahead of them.
                if b == len(bblocks) - 1:
                    w1q_sb = wpool.tile([128, KT, H], F8, tag="w",
                                        name="w1q_sb")
                    nc.sync.dma_start(w1q_sb[:], w1q_r[:])
                    w2q_sb = wpool.tile([128, HT, D], F8, tag="w",
                                        name="w2q_sb")
                    nc.sync.dma_start(w2q_sb[:], w2q_r[:])
                S_ = blk // 128
                gstep = 1 if b == len(bblocks) - 1 else 2
                for g in range(0, S_, gstep):
                    gs = min(gstep, S_ - g)
                    ob = opool.tile([128, 2, D], F32, tag="ob",
                                    name=f"bob_{b}_{g}")
                    for j in range(gs):
                        ts_ = g + j
                        tok_sl = bass.ds(ts_ * 128, 128)
                        ps2 = [ps2p.tile([128, 512], F32, tag="ps2",
                                         name=f"bps2_{b}_{ts_}_{i}")
                               for i in range(D // 512)]
                        for kh in range(HT):
                            for dc in range(D // 512):
                                nc.tensor.matmul(
                                    ps2[dc][:], hts[kh][:, tok_sl],
                                    w2b_sb[:, kh, dc * 512:(dc + 1) * 512],
                                    start=(kh == 0), stop=(kh == HT - 1))
                        tok_i = tok0 // 128 + ts_
                        for dc in range(D // 512):
                            nc.vector.tensor_scalar_mul(
                                ob[:, j, dc * 512:(dc + 1) * 512],
                                ps2[dc][:], cwb_sb[:, tok_i:tok_i + 1])
                    nc.sync.dma_start(
                        y_o[tok0 + g * 128:tok0 + (g + gs) * 128, :]
                        .rearrange("(s p) d -> p s d", p=128),
                        ob[:, 0:gs, :])
                tok0 += blk

            if not CB:
                w1q_sb = wpool.tile([128, KT, H], F8, tag="w", name="w1q_sb")
                nc.sync.dma_start(w1q_sb[:], w1q_r[:])
                w2q_sb = wpool.tile([128, HT, D], F8, tag="w", name="w2q_sb")
                nc.sync.dma_start(w2q_sb[:], w2q_r[:])

            # ---- fp8 section: e4m3 DoubleRow matmuls ----
            tok0 = 0
            for b, blk in enumerate(fblocks):
                xq = fxs[b]
                hps = []
                for ht in range(HT):
                    ps = ps1p.tile([128, blk], F32, tag="ps1",
                                   name=f"fps1_{b}_{ht}")
                    for kp in range(KT // 2):
                        nc.tensor.matmul(
                            ps[:],
                            w1q_sb[:, 2 * kp:2 * kp + 2,
                                   ht * 128:(ht + 1) * 128],
                            xq[:, 2 * kp:2 * kp + 2, :],
                            start=(kp == 0), stop=(kp == KT // 2 - 1),
                            perf_mode=DR)
                    if ht % 2 == 0:
                        hp = hpool.tile([128, 2, blk], F8, tag="h",
                                        name=f"fh_{b}_{ht // 2}")
                        hps.append(hp)
                    hf = hfpool.tile([128, blk], F32, tag="hf",
                                     name=f"hf_{b}_{ht}")
                    nc.scalar.activation(hf[:], ps[:], act,
                                         bias=b1_sb[:, ht:ht + 1],
                                         scale=1.0 / W1_SCALE)
                    nc.vector.tensor_scalar(hps[-1][:, ht % 2, :], hf[:],
                                            -H_CENTER, None, ALU.add)
                S_ = blk // 128
                for g in range(0, S_, 2):
                    gs = min(2, S_ - g)
                    ob = opool.tile([128, 2, D], F32, tag="ob",
                                    name=f"fob_{b}_{g}")
                    for j in range(gs):
                        ts_ = g + j
                        tok_sl = bass.ds(ts_ * 128, 128)
                        ps2 = [ps2p.tile([128, 512], F32, tag="ps2",
                                         name=f"fps2_{b}_{ts_}_{i}")
                               for i in range(D // 512)]
                        for khp in range(HT // 2):
                            for dc in range(D // 512):
                                nc.tensor.matmul(
                                    ps2[dc][:], hps[khp][:, :, tok_sl],
                                    w2q_sb[:, 2 * khp:2 * khp + 2,
                                           dc * 512:(dc + 1) * 512],
                                    start=(khp == 0),
                                    stop=(khp == HT // 2 - 1),
                                    perf_mode=DR)
                        tok_i = tok0 // 128 + ts_
                        for dc in range(D // 512):
                            nc.vector.tensor_scalar_mul(
                                ob[:, j, dc * 512:(dc + 1) * 512],
                                ps2[dc][:], cw8_sb[:, tok_i:tok_i + 1])
                    nc.sync.dma_start(
                        y_o[CB + tok0 + g * 128:CB + tok0 + (g + gs) * 128, :]
                        .rearrange("(s p) d -> p s d", p=128),
                        ob[:, 0:gs, :])
                tok0 += blk

    nc.compile()
    return nc


# ---------------------------------------------------------------- host
def kernel(x, gate_w, w1, b1, w2, b2, gamma, beta):
    x = np.asarray(x, dtype=np.float32)
    gate_w = np.asarray(gate_w, dtype=np.float32)
    w1 = np.asarray(w1, dtype=np.float32)
    b1 = np.asarray(b1, dtype=np.float32)
    w2 = np.asarray(w2, dtype=np.float32)
    b2 = np.asarray(b2, dtype=np.float32)
    gamma = np.asarray(gamma, dtype=np.float32)
    beta = np.asarray(beta, dtype=np.float32)

    xt = np.ascontiguousarray(x.reshape(T, D))

    # ---- phase 1: router/LN statistics on device ----
    if "p1" not in _cache:
        _cache["p1"] = build_phase1()
    nc1 = _cache["p1"]
    geff = gate_w * gamma[:, None]
    geff16 = geff.astype(BFNP).astype(np.float32)
    gate9 = np.concatenate([geff16, np.ones((D, 2), np.float32)],
                           axis=1).astype(BFNP)
    gate9_r = np.ascontiguousarray(
        gate9.reshape(KT, 128, 10).transpose(1, 0, 2))
    in1 = [{"xT": xt[c * TC:(c + 1) * TC].T.astype(BFNP),
            "gate9": gate9_r} for c in range(N_CORES)]
    res1 = run_bass_kernel_spmd(nc1, in1, list(range(N_CORES)))
    LAST_RESULTS["p1"] = res1
    stats = np.concatenate([res1.results[c]["stats"] for c in range(N_CORES)],
                           axis=0)                       # [T, 10]

    # ---- host: LN scalars + exact softmax/top-2 routing ----
    A = stats[:, :E].astype(np.float64)
    s1 = stats[:, E].astype(np.float64)
    s2 = stats[:, E + 2].astype(np.float64)
    mu = s1 / D
    varr = np.maximum(s2 / D - mu * mu, 0.0)
    rstd = 1.0 / np.sqrt(varr + LN_EPS)
    beta_row = beta.astype(np.float64) @ gate_w.astype(np.float64)
    colsum = geff16.sum(0, dtype=np.float64)
    logits = (A * rstd[:, None] - (mu * rstd)[:, None] * colsum[None, :]
              + beta_row[None, :])

    xn = (xt - mu.astype(np.float32)[:, None]) * rstd.astype(np.float32)[:, None]

    # The device logits carry bf16 noise; the top-2 SET is only at risk
    # where the 2nd/3rd logits are close. Recheck those tokens with an
    # exact f64 LayerNorm + router (mirrors the reference arithmetic).
    ls = np.sort(logits, axis=-1)
    flagged = (ls[:, -2] - ls[:, -3]) < 0.08
    if flagged.any():
        xfl = xt[flagged].astype(np.float64)
        muf = xfl.mean(-1, keepdims=True)
        varf = ((xfl - muf) ** 2).mean(-1, keepdims=True)
        xnf = ((xfl - muf) / np.sqrt(varf + LN_EPS)).astype(np.float32)
        logits[flagged] = (xnf.astype(np.float64) @ geff.astype(np.float64)
                           + beta_row[None, :])
        xn[flagged] = xnf

    top2 = np.argsort(-logits, axis=-1, kind="stable")[:, :TOP_K]
    # Renormalized top-2 weights depend only on the two selected logits:
    # compute those two dot products exactly so cw matches the reference.
    g_sel = geff.astype(np.float64).T[top2]                  # [T, 2, D]
    l_sel = (np.einsum("td,tkd->tk", xn.astype(np.float64), g_sel)
             + beta_row[top2])
    wts = 1.0 / (1.0 + np.exp(-(l_sel - l_sel[:, ::-1])))
    cwf = np.zeros((T, E), np.float32)
    np.put_along_axis(cwf, top2, wts.astype(np.float32), axis=-1)

    affine = not (np.all(gamma == 1.0) and np.all(beta == 0.0))
    if affine:
        xn = xn * gamma[None, :] + beta[None, :]

    # ---- host dispatch: per-expert slots sorted ascending by cw ----
    idxs = []
    for e in range(E):
        ix = np.nonzero(cwf[:, e])[0]
        order = np.argsort(cwf[ix, e], kind="stable")
        idxs.append(ix[order])
    counts = [len(ix) for ix in idxs]
    C = max(128, ((max(counts) + 127) // 128) * 128)
    S = int(round(C * FP8_FRAC / 128)) * 128
    S = max(0, min(S, C))
    CB = C - S

    key = ("p2", C, S)
    if key not in _cache:
        _cache[key] = build_phase2(C, S)
    nc2 = _cache[key]

    in2 = []
    f8_reals = []
    bf_reals = []
    for e in range(E):
        ix = idxs[e]
        npad = C - len(ix)
        nf8 = max(0, S - npad)          # real tokens in fp8 slots
        f8_tok = ix[:nf8]
        bf_tok = ix[nf8:]
        f8_reals.append(f8_tok)
        bf_reals.append(bf_tok)

        xq = np.zeros((S, D), dtype=E4NP)
        if len(f8_tok):
            xq[S - len(f8_tok):] = xn[f8_tok].astype(E4NP)
        xb = np.zeros((CB, D), dtype=BFNP)
        if len(bf_tok):
            xb[CB - len(bf_tok):] = xn[bf_tok].astype(BFNP)
        cw8 = np.zeros((max(S, 128),), np.float32)
        if len(f8_tok):
            cw8[S - len(f8_tok):S] = cwf[f8_tok, e] / W2_SCALE
        cwb = np.zeros((max(CB, 128),), np.float32)
        if len(bf_tok):
            cwb[CB - len(bf_tok):CB] = cwf[bf_tok, e]
        in2.append({
            "xbT": np.ascontiguousarray(xb.T),
            "xqT": np.ascontiguousarray(xq.T),
            "w1b": np.ascontiguousarray(w1[e].astype(BFNP)),
            "w2b": np.ascontiguousarray(w2[e].astype(BFNP)),
            "w1q": np.ascontiguousarray((w1[e] * W1_SCALE).astype(E4NP)),
            "w2q": np.ascontiguousarray((w2[e] * W2_SCALE).astype(E4NP)),
            "b1r": np.ascontiguousarray(b1[e].reshape(HT, 128).T),
            "cwbr": np.ascontiguousarray(
                cwb.reshape(-1, 128).T[:, :max(CB // 128, 1)]),
            "cw8r": np.ascontiguousarray(
                cw8.reshape(-1, 128).T[:, :max(S // 128, 1)]),
        })
    res2 = run_bass_kernel_spmd(nc2, in2, list(range(N_CORES)))
    LAST_RESULTS["p2"] = res2

    # ---- host combine: scatter-add + residual (+ per-expert b2) ----
    out = xt.copy()
    b2_any = bool(np.any(b2))
    for e in range(E):
        y = res2.results[e]["y"]            # [C, D]
        f8_tok, bf_tok = f8_reals[e], bf_reals[e]
        if len(bf_tok):
            out[bf_tok] += y[CB - len(bf_tok):CB]
        if len(f8_tok):
            w2q_f32 = in2[e]["w2q"].astype(np.float32)
            corr = (H_CENTER / W2_SCALE) * w2q_f32.sum(0)
            out[f8_tok] += (y[C - len(f8_tok):C]
                            + cwf[f8_tok, e][:, None] * corr[None, :])
        if b2_any:
            if len(bf_tok):
                out[bf_tok] += cwf[bf_tok, e][:, None] * b2[e][None, :]
            if len(f8_tok):
                out[f8_tok] += cwf[f8_tok, e][:, None] * b2[e][None, :]
    return out.reshape(B, L, D)


# revision 16
# speedup vs baseline: 1.1382x; 1.1382x over previous
"""CityExpertMoE Trainium2 kernel — mixed fp8/bf16 expert-parallel design.

Phase 1 (data-parallel over tokens): per core, upload x^T for its 1024
tokens once; the PE computes A = x @ [gate*gamma | 1] (router logits
numerator + column sums) and s2 = colsum(x^2) with x-slabs stationary.
Host derives LayerNorm mu/rstd, exact softmax/top-2 routing (f64), and
combine weights.

Host dispatch: per expert, tokens sorted ascending by combine weight cw;
the lowest-cw slots (including zero padding) go to an fp8 section, the
highest-cw tokens to a bf16 section. Quantization error enters the
output scaled by cw, so fp8 e4m3 (DoubleRow, ~1.44x tensor rate) on
low-cw slots keeps total L2 error ~1.6e-2 while accelerating ~71% of
the FLOPs.

Phase 2 (expert-parallel): core e runs expert e's FFN. bf16 section
first (weights resident), then fp8 section whose weights reuse the
bf16 weight SBUF slots (tag ring, WAR-tracked). Host combine:
scatter-add + residual (+ cw*b2 when b2 nonzero).
"""

import sys
import types

import numpy as np
import ml_dtypes

# If BASS_TRACE is set but the axon NTFF hook shim is absent, bass_utils
# would fail importing antenv.axon_hooks; register a no-op fallback.
try:
    import antenv.axon_hooks  # noqa: F401
except ImportError:
    _m = types.ModuleType("antenv.axon_hooks")
    _m._hook = None
    _m.set_axon_ntff_profile_hook = lambda h: setattr(_m, "_hook", h)
    _m.get_axon_ntff_profile_hook = lambda: _m._hook
    sys.modules["antenv.axon_hooks"] = _m
    try:
        import antenv
        antenv.axon_hooks = _m
    except ImportError:
        pass

import concourse.bass as bass
import concourse.mybir as mybir
import concourse.tile as tile
from concourse import bacc
from concourse.bass_utils import run_bass_kernel_spmd

F32 = mybir.dt.float32
F32R = mybir.dt.float32r
BF16 = mybir.dt.bfloat16
F8 = mybir.dt.float8e4
AF = mybir.ActivationFunctionType
ALU = mybir.AluOpType
DR = mybir.MatmulPerfMode.DoubleRow

E4NP = ml_dtypes.float8_e4m3
BFNP = ml_dtypes.bfloat16

B, L, D, H, E, TOP_K = 4, 2048, 1024, 4096, 8, 2
T = B * L               # 8192 tokens total
N_CORES = 8
TC = T // N_CORES       # 1024 tokens per core in phase 1
KT = D // 128           # 8 k-tiles over D
HT = H // 128           # 32 k-tiles over H
LN_EPS = 1e-5
W1_SCALE = 16.0         # w1 pre-scale before e4m3 cast (std -> ~0.5)
W2_SCALE = 32.0         # w2 pre-scale before e4m3 cast
FP8_FRAC = 0.70588      # fraction of slots (lowest cw) in the fp8 section

_cache: dict = {}
LAST_RESULTS: dict = {}


# ---------------------------------------------------------------- phase 1
def build_phase1():
    """Router/LN statistics: A = x @ [g|1|1] and s2 = colsum(x^2), bf16.

    x^T [D, TC] uploads once in bf16 (2MB): bf16 stationaries get the
    fast weight-load path, unlike fp32/f32r whose weight port runs 4x
    slower. The resulting ~0.6% logit noise is repaired on the host by
    an exact recheck of borderline tokens plus exact top-2 pair logits.
    Output stats [TC, 12] = [A(8) | s1 | s1 | s2 | s2]."""
    nc = bacc.Bacc("TRN2", target_bir_lowering=False, debug=False,
                   num_devices=N_CORES)
    xT_d = nc.dram_tensor("xT", [D, TC], BF16, kind="ExternalInput").ap()
    # 10 columns: 8 gate + 2 ones (even sizes keep every matmul ISA-legal)
    gate9_d = nc.dram_tensor("gate9", [128, KT, 10], BF16,
                             kind="ExternalInput").ap()
    stats_o = nc.dram_tensor("stats", [TC, 12], F32, kind="ExternalOutput").ap()
    NTT = TC // 128

    with tile.TileContext(nc) as tc:
        import contextlib
        with contextlib.ExitStack() as ctx:
            const = ctx.enter_context(tc.tile_pool(name="const", bufs=1))
            big = ctx.enter_context(tc.tile_pool(name="big", bufs=1))
            outp = ctx.enter_context(tc.tile_pool(name="outp", bufs=1))
            psp = ctx.enter_context(
                tc.tile_pool(name="psp", bufs=4, space="PSUM"))

            gate9 = const.tile([128, KT, 10], BF16)
            nc.sync.dma_start(gate9[:], gate9_d[:])

            xT_sb = big.tile([128, KT, TC], BF16)
            sq = big.tile([128, KT, TC], BF16)
            xT_r = xT_d.rearrange("(k p) t -> p k t", p=128)
            for ch in range(2):
                csl = bass.ts(ch, TC // 2)
                nc.sync.dma_start(xT_sb[:, :, csl], xT_r[:, :, csl])
                nc.vector.tensor_mul(sq[:, :, csl], xT_sb[:, :, csl],
                                     xT_sb[:, :, csl])

            stats = outp.tile([128, NTT, 12], F32)
            for t in range(NTT):
                tsl = bass.ts(t, 128)
                pA = psp.tile([128, 10], F32, tag="pA", name=f"pA_{t}")
                for k in range(KT):
                    nc.tensor.matmul(pA[:], xT_sb[:, k, tsl], gate9[:, k, :],
                                     start=(k == 0), stop=(k == KT - 1))
                nc.vector.tensor_copy(stats[:, t, 0:10], pA[:])
                pS = psp.tile([128, 2], F32, tag="pS", name=f"pS_{t}")
                for k in range(KT):
                    # gate9 cols 8:10 are all-ones summing vectors
                    nc.tensor.matmul(pS[:], sq[:, k, tsl], gate9[:, k, 8:10],
                                     start=(k == 0), stop=(k == KT - 1))
                nc.vector.tensor_copy(stats[:, t, 10:12], pS[:])
            nc.sync.dma_start(stats_o.rearrange("(t p) n -> p t n", p=128),
                              stats[:])

    nc.compile()
    return nc


def _blocks_bf16(n):
    """512-token blocks; small tails rebalanced like the baseline."""
    blocks = [512] * (n // 512)
    r = n % 512
    if r:
        if r < 256 and blocks:
            blocks.pop()
            total = 512 + r
            first = ((total + 1) // 2 + 127) // 128 * 128
            blocks.extend([first, total - first])
        else:
            blocks.append(r)
    return blocks


def _blocks_fp8(n):
    blocks = [512] * (n // 512)
    r = n % 512
    if r:
        blocks.append(r)
    return blocks


# ---------------------------------------------------------------- phase 2
def build_phase2(C: int, S: int, act=AF.Gelu):
    """Expert FFN on C slots: CB=C-S bf16 tokens then S fp8 tokens.

    y rows: [0, CB) = bf16 section slots, [CB, C) = fp8 section slots.
    fp8 weights are pre-scaled on host (W1_SCALE/W2_SCALE); the gelu
    activation un-scales mm1 (scale=1/W1_SCALE) and cw8r carries
    cw/W2_SCALE so mm2's scale folds into the existing combine mult."""
    CB = C - S
    bblocks = _blocks_bf16(CB)
    fblocks = _blocks_fp8(S)
    nc = bacc.Bacc("TRN2", target_bir_lowering=False, debug=False,
                   num_devices=N_CORES)
    xbT = nc.dram_tensor("xbT", [D, max(CB, 1)], BF16,
                         kind="ExternalInput").ap()
    xqT = nc.dram_tensor("xqT", [D, max(S, 1)], F8, kind="ExternalInput").ap()
    w1b_d = nc.dram_tensor("w1b", [D, H], BF16, kind="ExternalInput").ap()
    w2b_d = nc.dram_tensor("w2b", [H, D], BF16, kind="ExternalInput").ap()
    w1q_d = nc.dram_tensor("w1q", [D, H], F8, kind="ExternalInput").ap()
    w2q_d = nc.dram_tensor("w2q", [H, D], F8, kind="ExternalInput").ap()
    b1r_d = nc.dram_tensor("b1r", [128, HT], F32, kind="ExternalInput").ap()
    cwbr_d = nc.dram_tensor("cwbr", [128, max(CB // 128, 1)], F32,
                            kind="ExternalInput").ap()
    cw8r_d = nc.dram_tensor("cw8r", [128, max(S // 128, 1)], F32,
                            kind="ExternalInput").ap()
    y_o = nc.dram_tensor("y", [C, D], F32, kind="ExternalOutput").ap()

    xbT_r = xbT.rearrange("(k p) t -> p k t", p=128)
    xqT_r = xqT.rearrange("(k p) t -> p k t", p=128)
    w1b_r = w1b_d.rearrange("(k p) h -> p k h", p=128)
    w2b_r = w2b_d.rearrange("(k p) d -> p k d", p=128)
    w1q_r = w1q_d.rearrange("(k p) h -> p k h", p=128)
    w2q_r = w2q_d.rearrange("(k p) d -> p k d", p=128)

    with tile.TileContext(nc) as tc:
        import contextlib
        with contextlib.ExitStack() as ctx:
            const = ctx.enter_context(tc.tile_pool(name="const", bufs=1))
            wpool = ctx.enter_context(tc.tile_pool(name="w", bufs=2))
            xpool = ctx.enter_context(tc.tile_pool(name="xp", bufs=2))
            hpool = ctx.enter_context(tc.tile_pool(name="h", bufs=34))
            opool = ctx.enter_context(tc.tile_pool(name="o", bufs=2))
            ps1p = ctx.enter_context(
                tc.tile_pool(name="ps1", bufs=4, space="PSUM"))
            ps2p = ctx.enter_context(
                tc.tile_pool(name="ps2", bufs=4, space="PSUM"))

            # ---- DMA preamble: bf16 weights + block-0 activations first;
            # same issue-order trick as the baseline (block-0 x, then w1 in
            # chunks so mm1 can start early, then the rest).
            if CB:
                xb0 = xpool.tile([128, KT, bblocks[0]], BF16, tag="xb",
                                 name="xb_0")
                nc.sync.dma_start(xb0[:], xbT_r[:, :, 0:bblocks[0]])
            w1b_sb = wpool.tile([128, KT, H], BF16, tag="w", name="w1b_sb")
            w2b_sb = wpool.tile([128, HT, D], BF16, tag="w", name="w2b_sb")
            if CB:
                nc.sync.dma_start(w1b_sb[:, :, 0:H // 16],
                                  w1b_r[:, :, 0:H // 16])
            b1_sb = const.tile([128, HT], F32)
            nc.sync.dma_start(b1_sb[:], b1r_d[:])
            cwb_sb = const.tile([128, max(CB // 128, 1)], F32)
            nc.sync.dma_start(cwb_sb[:], cwbr_d[:])
            cw8_sb = const.tile([128, max(S // 128, 1)], F32)
            nc.sync.dma_start(cw8_sb[:], cw8r_d[:])
            if CB:
                nc.sync.dma_start(w1b_sb[:, :, H // 16:H // 8],
                                  w1b_r[:, :, H // 16:H // 8])
                for q in range(1, 8):
                    qsl = bass.ts(q, H // 8)
                    nc.sync.dma_start(w1b_sb[:, :, qsl], w1b_r[:, :, qsl])
                nc.sync.dma_start(w2b_sb[:, 0:HT // 2, :],
                                  w2b_r[:, 0:HT // 2, :])
                nc.sync.dma_start(w2b_sb[:, HT // 2:HT, :],
                                  w2b_r[:, HT // 2:HT, :])
            # fp8 activations are small (~1.25MB); queue them early so the
            # fp8 section never waits on them.
            fxs = []
            tok0 = 0
            for b, blk in enumerate(fblocks):
                xq = xpool.tile([128, KT, blk], F8, tag="xq", name=f"xq_{b}")
                nc.sync.dma_start(xq[:], xqT_r[:, :, tok0:tok0 + blk])
                fxs.append(xq)
                tok0 += blk

            # ---- bf16 section (baseline structure) ----
            tok0 = 0
            for b, blk in enumerate(bblocks):
                if b == 0:
                    xb = xb0
                else:
                    xb = xpool.tile([128, KT, blk], BF16, tag="xb",
                                    name=f"xb_{b}")
                    nc.sync.dma_start(xb[:], xbT_r[:, :, tok0:tok0 + blk])
                hts = []
                for ht in range(HT):
                    ps = ps1p.tile([128, blk], F32, tag="ps1",
                                   name=f"bps1_{b}_{ht}")
                    for k in range(KT):
                        nc.tensor.matmul(
                            ps[:], w1b_sb[:, k, ht * 128:(ht + 1) * 128],
                            xb[:, k, :], start=(k == 0), stop=(k == KT - 1))
                    htile = hpool.tile([128, blk], BF16, tag="h",
                                       name=f"bht_{b}_{ht}")
                    nc.scalar.activation(htile[:], ps[:], act,
                                         bias=b1_sb[:, ht:ht + 1])
                    hts.append(htile)
                # issue the fp8-weight DMAs right after the LAST bf16 mm1:
                # their SBUF slots (w-tag ring) free exactly then, and all
                # earlier y-outs are already queued ahead of them.
                if b == len(bblocks) - 1:
                    w1q_sb = wpool.tile([128, KT, H], F8, tag="w",
                                        name="w1q_sb")
                    nc.sync.dma_start(w1q_sb[:], w1q_r[:])
                    w2q_sb = wpool.tile([128, HT, D], F8, tag="w",
                                        name="w2q_sb")
                    nc.sync.dma_start(w2q_sb[:], w2q_r[:])
                S_ = blk // 128
                gstep = 1 if b == len(bblocks) - 1 else 2
                for g in range(0, S_, gstep):
                    gs = min(gstep, S_ - g)
                    ob = opool.tile([128, 2, D], F32, tag="ob",
                                    name=f"bob_{b}_{g}")
                    for j in range(gs):
                        ts_ = g + j
                        tok_sl = bass.ds(ts_ * 128, 128)
                        ps2 = [ps2p.tile([128, 512], F32, tag="ps2",
                                         name=f"bps2_{b}_{ts_}_{i}")
                               for i in range(D // 512)]
                        for kh in range(HT):
                            for dc in range(D // 512):
                                nc.tensor.matmul(
                                    ps2[dc][:], hts[kh][:, tok_sl],
                                    w2b_sb[:, kh, dc * 512:(dc + 1) * 512],
                                    start=(kh == 0), stop=(kh == HT - 1))
                        tok_i = tok0 // 128 + ts_
                        for dc in range(D // 512):
                            nc.vector.tensor_scalar_mul(
                                ob[:, j, dc * 512:(dc + 1) * 512],
                                ps2[dc][:], cwb_sb[:, tok_i:tok_i + 1])
                    nc.sync.dma_start(
                        y_o[tok0 + g * 128:tok0 + (g + gs) * 128, :]
                        .rearrange("(s p) d -> p s d", p=128),
                        ob[:, 0:gs, :])
                tok0 += blk

            if not CB:
                w1q_sb = wpool.tile([128, KT, H], F8, tag="w", name="w1q_sb")
                nc.sync.dma_start(w1q_sb[:], w1q_r[:])
                w2q_sb = wpool.tile([128, HT, D], F8, tag="w", name="w2q_sb")
                nc.sync.dma_start(w2q_sb[:], w2q_r[:])

            # ---- fp8 section: e4m3 DoubleRow matmuls ----
            tok0 = 0
            for b, blk in enumerate(fblocks):
                xq = fxs[b]
                hps = []
                for ht in range(HT):
                    ps = ps1p.tile([128, blk], F32, tag="ps1",
                                   name=f"fps1_{b}_{ht}")
                    for kp in range(KT // 2):
                        nc.tensor.matmul(
                            ps[:],
                            w1q_sb[:, 2 * kp:2 * kp + 2,
                                   ht * 128:(ht + 1) * 128],
                            xq[:, 2 * kp:2 * kp + 2, :],
                            start=(kp == 0), stop=(kp == KT // 2 - 1),
                            perf_mode=DR)
                    if ht % 2 == 0:
                        hp = hpool.tile([128, 2, blk], F8, tag="h",
                                        name=f"fh_{b}_{ht // 2}")
                        hps.append(hp)
                    nc.scalar.activation(hps[-1][:, ht % 2, :], ps[:],
                                         act, bias=b1_sb[:, ht:ht + 1],
                                         scale=1.0 / W1_SCALE)
                S_ = blk // 128
                for g in range(0, S_, 2):
                    gs = min(2, S_ - g)
                    ob = opool.tile([128, 2, D], F32, tag="ob",
                                    name=f"fob_{b}_{g}")
                    for j in range(gs):
                        ts_ = g + j
                        tok_sl = bass.ds(ts_ * 128, 128)
                        ps2 = [ps2p.tile([128, 512], F32, tag="ps2",
                                         name=f"fps2_{b}_{ts_}_{i}")
                               for i in range(D // 512)]
                        for khp in range(HT // 2):
                            for dc in range(D // 512):
                                nc.tensor.matmul(
                                    ps2[dc][:], hps[khp][:, :, tok_sl],
                                    w2q_sb[:, 2 * khp:2 * khp + 2,
                                           dc * 512:(dc + 1) * 512],
                                    start=(khp == 0),
                                    stop=(khp == HT // 2 - 1),
                                    perf_mode=DR)
                        tok_i = tok0 // 128 + ts_
                        for dc in range(D // 512):
                            nc.vector.tensor_scalar_mul(
                                ob[:, j, dc * 512:(dc + 1) * 512],
                                ps2[dc][:], cw8_sb[:, tok_i:tok_i + 1])
                    nc.sync.dma_start(
                        y_o[CB + tok0 + g * 128:CB + tok0 + (g + gs) * 128, :]
                        .rearrange("(s p) d -> p s d", p=128),
                        ob[:, 0:gs, :])
                tok0 += blk

    nc.compile()
    return nc


# ---------------------------------------------------------------- host
def kernel(x, gate_w, w1, b1, w2, b2, gamma, beta):
    x = np.asarray(x, dtype=np.float32)
    gate_w = np.asarray(gate_w, dtype=np.float32)
    w1 = np.asarray(w1, dtype=np.float32)
    b1 = np.asarray(b1, dtype=np.float32)
    w2 = np.asarray(w2, dtype=np.float32)
    b2 = np.asarray(b2, dtype=np.float32)
    gamma = np.asarray(gamma, dtype=np.float32)
    beta = np.asarray(beta, dtype=np.float32)

    xt = np.ascontiguousarray(x.reshape(T, D))

    # ---- phase 1: router/LN statistics on device ----
    if "p1" not in _cache:
        _cache["p1"] = build_phase1()
    nc1 = _cache["p1"]
    geff = gate_w * gamma[:, None]
    geff16 = geff.astype(BFNP).astype(np.float32)
    gate9 = np.concatenate([geff16, np.ones((D, 2), np.float32)],
                           axis=1).astype(BFNP)
    gate9_r = np.ascontiguousarray(
        gate9.reshape(KT, 128, 10).transpose(1, 0, 2))
    in1 = [{"xT": xt[c * TC:(c + 1) * TC].T.astype(BFNP),
            "gate9": gate9_r} for c in range(N_CORES)]
    res1 = run_bass_kernel_spmd(nc1, in1, list(range(N_CORES)))
    LAST_RESULTS["p1"] = res1
    stats = np.concatenate([res1.results[c]["stats"] for c in range(N_CORES)],
                           axis=0)                       # [T, 10]

    # ---- host: LN scalars + exact softmax/top-2 routing ----
    A = stats[:, :E].astype(np.float64)
    s1 = stats[:, E].astype(np.float64)
    s2 = stats[:, E + 2].astype(np.float64)
    mu = s1 / D
    varr = np.maximum(s2 / D - mu * mu, 0.0)
    rstd = 1.0 / np.sqrt(varr + LN_EPS)
    beta_row = beta.astype(np.float64) @ gate_w.astype(np.float64)
    colsum = geff16.sum(0, dtype=np.float64)
    logits = (A * rstd[:, None] - (mu * rstd)[:, None] * colsum[None, :]
              + beta_row[None, :])

    xn = (xt - mu.astype(np.float32)[:, None]) * rstd.astype(np.float32)[:, None]

    # The device logits carry bf16 noise; the top-2 SET is only at risk
    # where the 2nd/3rd logits are close. Recheck those tokens with an
    # exact f64 LayerNorm + router (mirrors the reference arithmetic).
    ls = np.sort(logits, axis=-1)
    flagged = (ls[:, -2] - ls[:, -3]) < 0.08
    if flagged.any():
        xfl = xt[flagged].astype(np.float64)
        muf = xfl.mean(-1, keepdims=True)
        varf = ((xfl - muf) ** 2).mean(-1, keepdims=True)
        xnf = ((xfl - muf) / np.sqrt(varf + LN_EPS)).astype(np.float32)
        logits[flagged] = (xnf.astype(np.float64) @ geff.astype(np.float64)
                           + beta_row[None, :])
        xn[flagged] = xnf

    top2 = np.argsort(-logits, axis=-1, kind="stable")[:, :TOP_K]
    # Renormalized top-2 weights depend only on the two selected logits:
    # compute those two dot products exactly so cw matches the reference.
    g_sel = geff.astype(np.float64).T[top2]                  # [T, 2, D]
    l_sel = (np.einsum("td,tkd->tk", xn.astype(np.float64), g_sel)
             + beta_row[top2])
    wts = 1.0 / (1.0 + np.exp(-(l_sel - l_sel[:, ::-1])))
    cwf = np.zeros((T, E), np.float32)
    np.put_along_axis(cwf, top2, wts.astype(np.float32), axis=-1)

    affine = not (np.all(gamma == 1.0) and np.all(beta == 0.0))
    if affine:
        xn = xn * gamma[None, :] + beta[None, :]

    # ---- host dispatch: per-expert slots sorted ascending by cw ----
    idxs = []
    for e in range(E):
        ix = np.nonzero(cwf[:, e])[0]
        order = np.argsort(cwf[ix, e], kind="stable")
        idxs.append(ix[order])
    counts = [len(ix) for ix in idxs]
    C = max(128, ((max(counts) + 127) // 128) * 128)
    S = int(round(C * FP8_FRAC / 128)) * 128
    S = max(0, min(S, C))
    CB = C - S

    key = ("p2", C, S)
    if key not in _cache:
        _cache[key] = build_phase2(C, S)
    nc2 = _cache[key]

    in2 = []
    f8_reals = []
    bf_reals = []
    for e in range(E):
        ix = idxs[e]
        npad = C - len(ix)
        nf8 = max(0, S - npad)          # real tokens in fp8 slots
        f8_tok = ix[:nf8]
        bf_tok = ix[nf8:]
        f8_reals.append(f8_tok)
        bf_reals.append(bf_tok)

        xq = np.zeros((S, D), dtype=E4NP)
        if len(f8_tok):
            xq[S - len(f8_tok):] = xn[f8_tok].astype(E4NP)
        xb = np.zeros((CB, D), dtype=BFNP)
        if len(bf_tok):
            xb[CB - len(bf_tok):] = xn[bf_tok].astype(BFNP)
        cw8 = np.zeros((max(S, 128),), np.float32)
        if len(f8_tok):
            cw8[S - len(f8_tok):S] = cwf[f8_tok, e] / W2_SCALE
        cwb = np.zeros((max(CB, 128),), np.float32)
        if len(bf_tok):
            cwb[CB - len(bf_tok):CB] = cwf[bf_tok, e]
        in2.append({
            "xbT": np.ascontiguousarray(xb.T),
            "xqT": np.ascontiguousarray(xq.T),
            "w1b": np.ascontiguousarray(w1[e].astype(BFNP)),
            "w2b": np.ascontiguousarray(w2[e].astype(BFNP)),
            "w1q": np.ascontiguousarray((w1[e] * W1_SCALE).astype(E4NP)),
            "w2q": np.ascontiguousarray((w2[e] * W2_SCALE).astype(E4NP)),
            "b1r": np.ascontiguousarray(b1[e].reshape(HT, 128).T),
            "cwbr": np.ascontiguousarray(
                cwb.reshape(-1, 128).T[:, :max(CB // 128, 1)]),
            "cw8r": np.ascontiguousarray(
                cw8.reshape(-1, 128).T[:, :max(S // 128, 1)]),
        })
    res2 = run_bass_kernel_spmd(nc2, in2, list(range(N_CORES)))
    LAST_RESULTS["p2"] = res2

    # ---- host combine: scatter-add + residual (+ per-expert b2) ----
    out = xt.copy()
    b2_any = bool(np.any(b2))
    for e in range(E):
        y = res2.results[e]["y"]            # [C, D]
        f8_tok, bf_tok = f8_reals[e], bf_reals[e]
        if len(bf_tok):
            out[bf_tok] += y[CB - len(bf_tok):CB]
        if len(f8_tok):
            out[f8_tok] += y[C - len(f8_tok):C]
        if b2_any:
            if len(bf_tok):
                out[bf_tok] += cwf[bf_tok, e][:, None] * b2[e][None, :]
            if len(f8_tok):
                out[f8_tok] += cwf[f8_tok, e][:, None] * b2[e][None, :]
    return out.reshape(B, L, D)


# revision 17
# speedup vs baseline: 1.1838x; 1.0400x over previous
"""CityExpertMoE Trainium2 kernel — mixed fp8/bf16 expert-parallel design.

Phase 1 (data-parallel over tokens): per core, upload x^T for its 1024
tokens once; the PE computes A = x @ [gate*gamma | 1] (router logits
numerator + column sums) and s2 = colsum(x^2) with x-slabs stationary.
Host derives LayerNorm mu/rstd, exact softmax/top-2 routing (f64), and
combine weights.

Host dispatch: per expert, tokens sorted ascending by combine weight cw;
the lowest-cw slots (including zero padding) go to an fp8 section, the
highest-cw tokens to a bf16 section. Quantization error enters the
output scaled by cw, so fp8 e4m3 (DoubleRow, ~1.44x tensor rate) on
low-cw slots keeps total L2 error ~1.6e-2 while accelerating ~71% of
the FLOPs.

Phase 2 (expert-parallel): core e runs expert e's FFN. bf16 section
first (weights resident), then fp8 section whose weights reuse the
bf16 weight SBUF slots (tag ring, WAR-tracked). Host combine:
scatter-add + residual (+ cw*b2 when b2 nonzero).
"""

import sys
import types

import numpy as np
import ml_dtypes

# If BASS_TRACE is set but the axon NTFF hook shim is absent, bass_utils
# would fail importing antenv.axon_hooks; register a no-op fallback.
try:
    import antenv.axon_hooks  # noqa: F401
except ImportError:
    _m = types.ModuleType("antenv.axon_hooks")
    _m._hook = None
    _m.set_axon_ntff_profile_hook = lambda h: setattr(_m, "_hook", h)
    _m.get_axon_ntff_profile_hook = lambda: _m._hook
    sys.modules["antenv.axon_hooks"] = _m
    try:
        import antenv
        antenv.axon_hooks = _m
    except ImportError:
        pass

import concourse.bass as bass
import concourse.mybir as mybir
import concourse.tile as tile
from concourse import bacc
from concourse.bass_utils import run_bass_kernel_spmd

F32 = mybir.dt.float32
F32R = mybir.dt.float32r
BF16 = mybir.dt.bfloat16
F8 = mybir.dt.float8e4
AF = mybir.ActivationFunctionType
ALU = mybir.AluOpType
DR = mybir.MatmulPerfMode.DoubleRow

E4NP = ml_dtypes.float8_e4m3
BFNP = ml_dtypes.bfloat16

B, L, D, H, E, TOP_K = 4, 2048, 1024, 4096, 8, 2
T = B * L               # 8192 tokens total
N_CORES = 8
TC = T // N_CORES       # 1024 tokens per core in phase 1
KT = D // 128           # 8 k-tiles over D
HT = H // 128           # 32 k-tiles over H
LN_EPS = 1e-5
W1_SCALE = 16.0         # w1 pre-scale before e4m3 cast (std -> ~0.5)
W2_SCALE = 32.0         # w2 pre-scale before e4m3 cast
FP8_FRAC = 0.76471      # fraction of slots (lowest cw) in the fp8 section

_cache: dict = {}
LAST_RESULTS: dict = {}


# ---------------------------------------------------------------- phase 1
def build_phase1():
    """Router/LN statistics: A = x @ [g|1|1] and s2 = colsum(x^2), bf16.

    x^T [D, TC] uploads once in bf16 (2MB): bf16 stationaries get the
    fast weight-load path, unlike fp32/f32r whose weight port runs 4x
    slower. The resulting ~0.6% logit noise is repaired on the host by
    an exact recheck of borderline tokens plus exact top-2 pair logits.
    Output stats [TC, 12] = [A(8) | s1 | s1 | s2 | s2]."""
    nc = bacc.Bacc("TRN2", target_bir_lowering=False, debug=False,
                   num_devices=N_CORES)
    xT_d = nc.dram_tensor("xT", [D, TC], BF16, kind="ExternalInput").ap()
    # 10 columns: 8 gate + 2 ones (even sizes keep every matmul ISA-legal)
    gate9_d = nc.dram_tensor("gate9", [128, KT, 10], BF16,
                             kind="ExternalInput").ap()
    stats_o = nc.dram_tensor("stats", [TC, 12], F32, kind="ExternalOutput").ap()
    NTT = TC // 128

    with tile.TileContext(nc) as tc:
        import contextlib
        with contextlib.ExitStack() as ctx:
            const = ctx.enter_context(tc.tile_pool(name="const", bufs=1))
            big = ctx.enter_context(tc.tile_pool(name="big", bufs=1))
            outp = ctx.enter_context(tc.tile_pool(name="outp", bufs=1))
            psp = ctx.enter_context(
                tc.tile_pool(name="psp", bufs=4, space="PSUM"))

            gate9 = const.tile([128, KT, 10], BF16)
            nc.sync.dma_start(gate9[:], gate9_d[:])

            xT_sb = big.tile([128, KT, TC], BF16)
            sq = big.tile([128, KT, TC], BF16)
            xT_r = xT_d.rearrange("(k p) t -> p k t", p=128)
            for ch in range(2):
                csl = bass.ts(ch, TC // 2)
                nc.sync.dma_start(xT_sb[:, :, csl], xT_r[:, :, csl])
                nc.vector.tensor_mul(sq[:, :, csl], xT_sb[:, :, csl],
                                     xT_sb[:, :, csl])

            stats = outp.tile([128, NTT, 12], F32)
            for t in range(NTT):
                tsl = bass.ts(t, 128)
                pA = psp.tile([128, 10], F32, tag="pA", name=f"pA_{t}")
                for k in range(KT):
                    nc.tensor.matmul(pA[:], xT_sb[:, k, tsl], gate9[:, k, :],
                                     start=(k == 0), stop=(k == KT - 1))
                nc.vector.tensor_copy(stats[:, t, 0:10], pA[:])
                pS = psp.tile([128, 2], F32, tag="pS", name=f"pS_{t}")
                for k in range(KT):
                    # gate9 cols 8:10 are all-ones summing vectors
                    nc.tensor.matmul(pS[:], sq[:, k, tsl], gate9[:, k, 8:10],
                                     start=(k == 0), stop=(k == KT - 1))
                nc.vector.tensor_copy(stats[:, t, 10:12], pS[:])
            nc.sync.dma_start(stats_o.rearrange("(t p) n -> p t n", p=128),
                              stats[:])

    nc.compile()
    return nc


def _blocks_bf16(n):
    """512-token blocks; small tails rebalanced like the baseline."""
    blocks = [512] * (n // 512)
    r = n % 512
    if r:
        if r < 256 and blocks:
            blocks.pop()
            total = 512 + r
            first = ((total + 1) // 2 + 127) // 128 * 128
            blocks.extend([first, total - first])
        else:
            blocks.append(r)
    return blocks


def _blocks_fp8(n):
    blocks = [512] * (n // 512)
    r = n % 512
    if r:
        blocks.append(r)
    return blocks


# ---------------------------------------------------------------- phase 2
def build_phase2(C: int, S: int, act=AF.Gelu):
    """Expert FFN on C slots: CB=C-S bf16 tokens then S fp8 tokens.

    y rows: [0, CB) = bf16 section slots, [CB, C) = fp8 section slots.
    fp8 weights are pre-scaled on host (W1_SCALE/W2_SCALE); the gelu
    activation un-scales mm1 (scale=1/W1_SCALE) and cw8r carries
    cw/W2_SCALE so mm2's scale folds into the existing combine mult."""
    CB = C - S
    bblocks = _blocks_bf16(CB)
    fblocks = _blocks_fp8(S)
    nc = bacc.Bacc("TRN2", target_bir_lowering=False, debug=False,
                   num_devices=N_CORES)
    xbT = nc.dram_tensor("xbT", [D, max(CB, 1)], BF16,
                         kind="ExternalInput").ap()
    xqT = nc.dram_tensor("xqT", [D, max(S, 1)], F8, kind="ExternalInput").ap()
    w1b_d = nc.dram_tensor("w1b", [D, H], BF16, kind="ExternalInput").ap()
    w2b_d = nc.dram_tensor("w2b", [H, D], BF16, kind="ExternalInput").ap()
    w1q_d = nc.dram_tensor("w1q", [D, H], F8, kind="ExternalInput").ap()
    w2q_d = nc.dram_tensor("w2q", [H, D], F8, kind="ExternalInput").ap()
    b1r_d = nc.dram_tensor("b1r", [128, HT], F32, kind="ExternalInput").ap()
    cwbr_d = nc.dram_tensor("cwbr", [128, max(CB // 128, 1)], F32,
                            kind="ExternalInput").ap()
    cw8r_d = nc.dram_tensor("cw8r", [128, max(S // 128, 1)], F32,
                            kind="ExternalInput").ap()
    y_o = nc.dram_tensor("y", [C, D], F32, kind="ExternalOutput").ap()

    xbT_r = xbT.rearrange("(k p) t -> p k t", p=128)
    xqT_r = xqT.rearrange("(k p) t -> p k t", p=128)
    w1b_r = w1b_d.rearrange("(k p) h -> p k h", p=128)
    w2b_r = w2b_d.rearrange("(k p) d -> p k d", p=128)
    w1q_r = w1q_d.rearrange("(k p) h -> p k h", p=128)
    w2q_r = w2q_d.rearrange("(k p) d -> p k d", p=128)

    with tile.TileContext(nc) as tc:
        import contextlib
        with contextlib.ExitStack() as ctx:
            const = ctx.enter_context(tc.tile_pool(name="const", bufs=1))
            wpool = ctx.enter_context(tc.tile_pool(name="w", bufs=2))
            xpool = ctx.enter_context(tc.tile_pool(name="xp", bufs=2))
            hpool = ctx.enter_context(tc.tile_pool(name="h", bufs=34))
            opool = ctx.enter_context(tc.tile_pool(name="o", bufs=2))
            ps1p = ctx.enter_context(
                tc.tile_pool(name="ps1", bufs=4, space="PSUM"))
            ps2p = ctx.enter_context(
                tc.tile_pool(name="ps2", bufs=4, space="PSUM"))

            # ---- DMA preamble: bf16 weights + block-0 activations first;
            # same issue-order trick as the baseline (block-0 x, then w1 in
            # chunks so mm1 can start early, then the rest).
            if CB:
                xb0 = xpool.tile([128, KT, bblocks[0]], BF16, tag="xb",
                                 name="xb_0",
                                 bufs=1 if len(bblocks) == 1 else None)
                nc.sync.dma_start(xb0[:], xbT_r[:, :, 0:bblocks[0]])
            w1b_sb = wpool.tile([128, KT, H], BF16, tag="w", name="w1b_sb")
            w2b_sb = wpool.tile([128, HT, D], BF16, tag="w", name="w2b_sb")
            if CB:
                nc.sync.dma_start(w1b_sb[:, :, 0:H // 16],
                                  w1b_r[:, :, 0:H // 16])
            b1_sb = const.tile([128, HT], F32)
            nc.sync.dma_start(b1_sb[:], b1r_d[:])
            cwb_sb = const.tile([128, max(CB // 128, 1)], F32)
            nc.sync.dma_start(cwb_sb[:], cwbr_d[:])
            cw8_sb = const.tile([128, max(S // 128, 1)], F32)
            nc.sync.dma_start(cw8_sb[:], cw8r_d[:])
            if CB:
                nc.sync.dma_start(w1b_sb[:, :, H // 16:H // 8],
                                  w1b_r[:, :, H // 16:H // 8])
                for q in range(1, 8):
                    qsl = bass.ts(q, H // 8)
                    nc.sync.dma_start(w1b_sb[:, :, qsl], w1b_r[:, :, qsl])
                nc.sync.dma_start(w2b_sb[:, 0:HT // 2, :],
                                  w2b_r[:, 0:HT // 2, :])
                nc.sync.dma_start(w2b_sb[:, HT // 2:HT, :],
                                  w2b_r[:, HT // 2:HT, :])
            # fp8 activations are small (~1.25MB); queue them early so the
            # fp8 section never waits on them.
            fxs = []
            tok0 = 0
            for b, blk in enumerate(fblocks):
                xq = xpool.tile([128, KT, blk], F8, tag="xq", name=f"xq_{b}")
                nc.sync.dma_start(xq[:], xqT_r[:, :, tok0:tok0 + blk])
                fxs.append(xq)
                tok0 += blk

            # ---- bf16 section (baseline structure) ----
            tok0 = 0
            for b, blk in enumerate(bblocks):
                if b == 0:
                    xb = xb0
                else:
                    xb = xpool.tile([128, KT, blk], BF16, tag="xb",
                                    name=f"xb_{b}")
                    nc.sync.dma_start(xb[:], xbT_r[:, :, tok0:tok0 + blk])
                hts = []
                for ht in range(HT):
                    ps = ps1p.tile([128, blk], F32, tag="ps1",
                                   name=f"bps1_{b}_{ht}")
                    for k in range(KT):
                        nc.tensor.matmul(
                            ps[:], w1b_sb[:, k, ht * 128:(ht + 1) * 128],
                            xb[:, k, :], start=(k == 0), stop=(k == KT - 1))
                    htile = hpool.tile([128, blk], BF16, tag="h",
                                       name=f"bht_{b}_{ht}")
                    nc.scalar.activation(htile[:], ps[:], act,
                                         bias=b1_sb[:, ht:ht + 1])
                    hts.append(htile)
                # issue the fp8-weight DMAs right after the LAST bf16 mm1:
                # their SBUF slots (w-tag ring) free exactly then, and all
                # earlier y-outs are already queued ahead of them.
                if b == len(bblocks) - 1:
                    w1q_sb = wpool.tile([128, KT, H], F8, tag="w",
                                        name="w1q_sb")
                    nc.sync.dma_start(w1q_sb[:], w1q_r[:])
                    w2q_sb = wpool.tile([128, HT, D], F8, tag="w",
                                        name="w2q_sb")
                    nc.sync.dma_start(w2q_sb[:], w2q_r[:])
                S_ = blk // 128
                gstep = 1 if b == len(bblocks) - 1 else 2
                for g in range(0, S_, gstep):
                    gs = min(gstep, S_ - g)
                    ob = opool.tile([128, 2, D], F32, tag="ob",
                                    name=f"bob_{b}_{g}")
                    for j in range(gs):
                        ts_ = g + j
                        tok_sl = bass.ds(ts_ * 128, 128)
                        ps2 = [ps2p.tile([128, 512], F32, tag="ps2",
                                         name=f"bps2_{b}_{ts_}_{i}")
                               for i in range(D // 512)]
                        for kh in range(HT):
                            for dc in range(D // 512):
                                nc.tensor.matmul(
                                    ps2[dc][:], hts[kh][:, tok_sl],
                                    w2b_sb[:, kh, dc * 512:(dc + 1) * 512],
                                    start=(kh == 0), stop=(kh == HT - 1))
                        tok_i = tok0 // 128 + ts_
                        for dc in range(D // 512):
                            nc.vector.tensor_scalar_mul(
                                ob[:, j, dc * 512:(dc + 1) * 512],
                                ps2[dc][:], cwb_sb[:, tok_i:tok_i + 1])
                    nc.sync.dma_start(
                        y_o[tok0 + g * 128:tok0 + (g + gs) * 128, :]
                        .rearrange("(s p) d -> p s d", p=128),
                        ob[:, 0:gs, :])
                tok0 += blk

            if not CB:
                w1q_sb = wpool.tile([128, KT, H], F8, tag="w", name="w1q_sb")
                nc.sync.dma_start(w1q_sb[:], w1q_r[:])
                w2q_sb = wpool.tile([128, HT, D], F8, tag="w", name="w2q_sb")
                nc.sync.dma_start(w2q_sb[:], w2q_r[:])

            # ---- fp8 section: e4m3 DoubleRow matmuls ----
            tok0 = 0
            for b, blk in enumerate(fblocks):
                xq = fxs[b]
                hps = []
                for ht in range(HT):
                    ps = ps1p.tile([128, blk], F32, tag="ps1",
                                   name=f"fps1_{b}_{ht}")
                    for kp in range(KT // 2):
                        nc.tensor.matmul(
                            ps[:],
                            w1q_sb[:, 2 * kp:2 * kp + 2,
                                   ht * 128:(ht + 1) * 128],
                            xq[:, 2 * kp:2 * kp + 2, :],
                            start=(kp == 0), stop=(kp == KT // 2 - 1),
                            perf_mode=DR)
                    if ht % 2 == 0:
                        hp = hpool.tile([128, 2, blk], F8, tag="h",
                                        name=f"fh_{b}_{ht // 2}")
                        hps.append(hp)
                    nc.scalar.activation(hps[-1][:, ht % 2, :], ps[:],
                                         act, bias=b1_sb[:, ht:ht + 1],
                                         scale=1.0 / W1_SCALE)
                S_ = blk // 128
                for g in range(0, S_, 2):
                    gs = min(2, S_ - g)
                    ob = opool.tile([128, 2, D], F32, tag="ob",
                                    name=f"fob_{b}_{g}")
                    for j in range(gs):
                        ts_ = g + j
                        tok_sl = bass.ds(ts_ * 128, 128)
                        ps2 = [ps2p.tile([128, 512], F32, tag="ps2",
                                         name=f"fps2_{b}_{ts_}_{i}")
                               for i in range(D // 512)]
                        for khp in range(HT // 2):
                            for dc in range(D // 512):
                                nc.tensor.matmul(
                                    ps2[dc][:], hps[khp][:, :, tok_sl],
                                    w2q_sb[:, 2 * khp:2 * khp + 2,
                                           dc * 512:(dc + 1) * 512],
                                    start=(khp == 0),
                                    stop=(khp == HT // 2 - 1),
                                    perf_mode=DR)
                        tok_i = tok0 // 128 + ts_
                        for dc in range(D // 512):
                            nc.vector.tensor_scalar_mul(
                                ob[:, j, dc * 512:(dc + 1) * 512],
                                ps2[dc][:], cw8_sb[:, tok_i:tok_i + 1])
                    nc.sync.dma_start(
                        y_o[CB + tok0 + g * 128:CB + tok0 + (g + gs) * 128, :]
                        .rearrange("(s p) d -> p s d", p=128),
                        ob[:, 0:gs, :])
                tok0 += blk

    nc.compile()
    return nc


# ---------------------------------------------------------------- host
def kernel(x, gate_w, w1, b1, w2, b2, gamma, beta):
    x = np.asarray(x, dtype=np.float32)
    gate_w = np.asarray(gate_w, dtype=np.float32)
    w1 = np.asarray(w1, dtype=np.float32)
    b1 = np.asarray(b1, dtype=np.float32)
    w2 = np.asarray(w2, dtype=np.float32)
    b2 = np.asarray(b2, dtype=np.float32)
    gamma = np.asarray(gamma, dtype=np.float32)
    beta = np.asarray(beta, dtype=np.float32)

    xt = np.ascontiguousarray(x.reshape(T, D))

    # ---- phase 1: router/LN statistics on device ----
    if "p1" not in _cache:
        _cache["p1"] = build_phase1()
    nc1 = _cache["p1"]
    geff = gate_w * gamma[:, None]
    geff16 = geff.astype(BFNP).astype(np.float32)
    gate9 = np.concatenate([geff16, np.ones((D, 2), np.float32)],
                           axis=1).astype(BFNP)
    gate9_r = np.ascontiguousarray(
        gate9.reshape(KT, 128, 10).transpose(1, 0, 2))
    in1 = [{"xT": xt[c * TC:(c + 1) * TC].T.astype(BFNP),
            "gate9": gate9_r} for c in range(N_CORES)]
    res1 = run_bass_kernel_spmd(nc1, in1, list(range(N_CORES)))
    LAST_RESULTS["p1"] = res1
    stats = np.concatenate([res1.results[c]["stats"] for c in range(N_CORES)],
                           axis=0)                       # [T, 10]

    # ---- host: LN scalars + exact softmax/top-2 routing ----
    A = stats[:, :E].astype(np.float64)
    s1 = stats[:, E].astype(np.float64)
    s2 = stats[:, E + 2].astype(np.float64)
    mu = s1 / D
    varr = np.maximum(s2 / D - mu * mu, 0.0)
    rstd = 1.0 / np.sqrt(varr + LN_EPS)
    beta_row = beta.astype(np.float64) @ gate_w.astype(np.float64)
    colsum = geff16.sum(0, dtype=np.float64)
    logits = (A * rstd[:, None] - (mu * rstd)[:, None] * colsum[None, :]
              + beta_row[None, :])

    xn = (xt - mu.astype(np.float32)[:, None]) * rstd.astype(np.float32)[:, None]

    # The device logits carry bf16 noise; the top-2 SET is only at risk
    # where the 2nd/3rd logits are close. Recheck those tokens with an
    # exact f64 LayerNorm + router (mirrors the reference arithmetic).
    ls = np.sort(logits, axis=-1)
    flagged = (ls[:, -2] - ls[:, -3]) < 0.08
    if flagged.any():
        xfl = xt[flagged].astype(np.float64)
        muf = xfl.mean(-1, keepdims=True)
        varf = ((xfl - muf) ** 2).mean(-1, keepdims=True)
        xnf = ((xfl - muf) / np.sqrt(varf + LN_EPS)).astype(np.float32)
        logits[flagged] = (xnf.astype(np.float64) @ geff.astype(np.float64)
                           + beta_row[None, :])
        xn[flagged] = xnf

    top2 = np.argsort(-logits, axis=-1, kind="stable")[:, :TOP_K]
    # Renormalized top-2 weights depend only on the two selected logits:
    # compute those two dot products exactly so cw matches the reference.
    g_sel = geff.astype(np.float64).T[top2]                  # [T, 2, D]
    l_sel = (np.einsum("td,tkd->tk", xn.astype(np.float64), g_sel)
             + beta_row[top2])
    wts = 1.0 / (1.0 + np.exp(-(l_sel - l_sel[:, ::-1])))
    cwf = np.zeros((T, E), np.float32)
    np.put_along_axis(cwf, top2, wts.astype(np.float32), axis=-1)

    affine = not (np.all(gamma == 1.0) and np.all(beta == 0.0))
    if affine:
        xn = xn * gamma[None, :] + beta[None, :]

    # ---- host dispatch: per-expert slots sorted ascending by cw ----
    idxs = []
    for e in range(E):
        ix = np.nonzero(cwf[:, e])[0]
        order = np.argsort(cwf[ix, e], kind="stable")
        idxs.append(ix[order])
    counts = [len(ix) for ix in idxs]
    C = max(128, ((max(counts) + 127) // 128) * 128)
    S = int(round(C * FP8_FRAC / 128)) * 128
    S = max(0, min(S, C))
    CB = C - S

    key = ("p2", C, S)
    if key not in _cache:
        _cache[key] = build_phase2(C, S)
    nc2 = _cache[key]

    in2 = []
    f8_reals = []
    bf_reals = []
    for e in range(E):
        ix = idxs[e]
        npad = C - len(ix)
        nf8 = max(0, S - npad)          # real tokens in fp8 slots
        f8_tok = ix[:nf8]
        bf_tok = ix[nf8:]
        f8_reals.append(f8_tok)
        bf_reals.append(bf_tok)

        xq = np.zeros((S, D), dtype=E4NP)
        if len(f8_tok):
            xq[S - len(f8_tok):] = xn[f8_tok].astype(E4NP)
        xb = np.zeros((CB, D), dtype=BFNP)
        if len(bf_tok):
            xb[CB - len(bf_tok):] = xn[bf_tok].astype(BFNP)
        cw8 = np.zeros((max(S, 128),), np.float32)
        if len(f8_tok):
            cw8[S - len(f8_tok):S] = cwf[f8_tok, e] / W2_SCALE
        cwb = np.zeros((max(CB, 128),), np.float32)
        if len(bf_tok):
            cwb[CB - len(bf_tok):CB] = cwf[bf_tok, e]
        in2.append({
            "xbT": np.ascontiguousarray(xb.T),
            "xqT": np.ascontiguousarray(xq.T),
            "w1b": np.ascontiguousarray(w1[e].astype(BFNP)),
            "w2b": np.ascontiguousarray(w2[e].astype(BFNP)),
            "w1q": np.ascontiguousarray((w1[e] * W1_SCALE).astype(E4NP)),
            "w2q": np.ascontiguousarray((w2[e] * W2_SCALE).astype(E4NP)),
            "b1r": np.ascontiguousarray(b1[e].reshape(HT, 128).T),
            "cwbr": np.ascontiguousarray(
                cwb.reshape(-1, 128).T[:, :max(CB // 128, 1)]),
            "cw8r": np.ascontiguousarray(
                cw8.reshape(-1, 128).T[:, :max(S // 128, 1)]),
        })
    res2 = run_bass_kernel_spmd(nc2, in2, list(range(N_CORES)))
    LAST_RESULTS["p2"] = res2

    # ---- host combine: scatter-add + residual (+ per-expert b2) ----
    out = xt.copy()
    b2_any = bool(np.any(b2))
    for e in range(E):
        y = res2.results[e]["y"]            # [C, D]
        f8_tok, bf_tok = f8_reals[e], bf_reals[e]
        if len(bf_tok):
            out[bf_tok] += y[CB - len(bf_tok):CB]
        if len(f8_tok):
            out[f8_tok] += y[C - len(f8_tok):C]
        if b2_any:
            if len(bf_tok):
                out[bf_tok] += cwf[bf_tok, e][:, None] * b2[e][None, :]
            if len(f8_tok):
                out[f8_tok] += cwf[f8_tok, e][:, None] * b2[e][None, :]
    return out.reshape(B, L, D)


# revision 19
# speedup vs baseline: 1.1859x; 1.0018x over previous
"""CityExpertMoE Trainium2 kernel — mixed fp8/bf16 expert-parallel design.

Phase 1 (data-parallel over tokens): per core, upload x^T for its 1024
tokens once; the PE computes A = x @ [gate*gamma | 1] (router logits
numerator + column sums) and s2 = colsum(x^2) with x-slabs stationary.
Host derives LayerNorm mu/rstd, exact softmax/top-2 routing (f64), and
combine weights.

Host dispatch: per expert, tokens sorted ascending by combine weight cw;
the lowest-cw slots (including zero padding) go to an fp8 section, the
highest-cw tokens to a bf16 section. Quantization error enters the
output scaled by cw, so fp8 e4m3 (DoubleRow, ~2x measured tensor rate)
on low-cw slots keeps total L2 error ~1.74e-2 (gate 2e-2) while
accelerating ~76% of the FLOPs.

Phase 2 (expert-parallel): core e runs expert e's FFN. bf16 section
first (weights resident), then fp8 section whose weights reuse the
bf16 weight SBUF slots (tag ring, WAR-tracked). Host combine:
scatter-add + residual (+ cw*b2 when b2 nonzero).
"""

import sys
import types

import numpy as np
import ml_dtypes

# If BASS_TRACE is set but the axon NTFF hook shim is absent, bass_utils
# would fail importing antenv.axon_hooks; register a no-op fallback.
try:
    import antenv.axon_hooks  # noqa: F401
except ImportError:
    _m = types.ModuleType("antenv.axon_hooks")
    _m._hook = None
    _m.set_axon_ntff_profile_hook = lambda h: setattr(_m, "_hook", h)
    _m.get_axon_ntff_profile_hook = lambda: _m._hook
    sys.modules["antenv.axon_hooks"] = _m
    try:
        import antenv
        antenv.axon_hooks = _m
    except ImportError:
        pass

import concourse.bass as bass
import concourse.mybir as mybir
import concourse.tile as tile
from concourse import bacc
from concourse.bass_utils import run_bass_kernel_spmd

F32 = mybir.dt.float32
F32R = mybir.dt.float32r
BF16 = mybir.dt.bfloat16
F8 = mybir.dt.float8e4
AF = mybir.ActivationFunctionType
ALU = mybir.AluOpType
DR = mybir.MatmulPerfMode.DoubleRow

E4NP = ml_dtypes.float8_e4m3
BFNP = ml_dtypes.bfloat16

B, L, D, H, E, TOP_K = 4, 2048, 1024, 4096, 8, 2
T = B * L               # 8192 tokens total
N_CORES = 8
TC = T // N_CORES       # 1024 tokens per core in phase 1
KT = D // 128           # 8 k-tiles over D
HT = H // 128           # 32 k-tiles over H
LN_EPS = 1e-5
W1_SCALE = 16.0         # w1 pre-scale before e4m3 cast (std -> ~0.5)
W2_SCALE = 32.0         # w2 pre-scale before e4m3 cast
FP8_FRAC = 0.76471      # fraction of slots (lowest cw) in the fp8 section

_cache: dict = {}
LAST_RESULTS: dict = {}


# ---------------------------------------------------------------- phase 1
def build_phase1():
    """Router/LN statistics: A = x @ [g|1|1] and s2 = colsum(x^2), bf16.

    x^T [D, TC] uploads once in bf16 (2MB): bf16 stationaries get the
    fast weight-load path, unlike fp32/f32r whose weight port runs 4x
    slower. The resulting ~0.6% logit noise is repaired on the host by
    an exact recheck of borderline tokens plus exact top-2 pair logits.
    Output stats [TC, 12] = [A(8) | s1 | s1 | s2 | s2]."""
    nc = bacc.Bacc("TRN2", target_bir_lowering=False, debug=False,
                   num_devices=N_CORES)
    xT_d = nc.dram_tensor("xT", [D, TC], BF16, kind="ExternalInput").ap()
    # 10 columns: 8 gate + 2 ones (even sizes keep every matmul ISA-legal)
    gate9_d = nc.dram_tensor("gate9", [128, KT, 10], BF16,
                             kind="ExternalInput").ap()
    stats_o = nc.dram_tensor("stats", [TC, 12], F32, kind="ExternalOutput").ap()
    NTT = TC // 128

    with tile.TileContext(nc) as tc:
        import contextlib
        with contextlib.ExitStack() as ctx:
            const = ctx.enter_context(tc.tile_pool(name="const", bufs=1))
            big = ctx.enter_context(tc.tile_pool(name="big", bufs=1))
            outp = ctx.enter_context(tc.tile_pool(name="outp", bufs=1))
            psp = ctx.enter_context(
                tc.tile_pool(name="psp", bufs=4, space="PSUM"))

            gate9 = const.tile([128, KT, 10], BF16)
            nc.sync.dma_start(gate9[:], gate9_d[:])

            xT_sb = big.tile([128, KT, TC], BF16)
            sq = big.tile([128, KT, TC], BF16)
            xT_r = xT_d.rearrange("(k p) t -> p k t", p=128)
            for ch in range(2):
                csl = bass.ts(ch, TC // 2)
                nc.sync.dma_start(xT_sb[:, :, csl], xT_r[:, :, csl])
                nc.vector.tensor_mul(sq[:, :, csl], xT_sb[:, :, csl],
                                     xT_sb[:, :, csl])

            stats = outp.tile([128, NTT, 12], F32)
            for t in range(NTT):
                tsl = bass.ts(t, 128)
                pA = psp.tile([128, 10], F32, tag="pA", name=f"pA_{t}")
                for k in range(KT):
                    nc.tensor.matmul(pA[:], xT_sb[:, k, tsl], gate9[:, k, :],
                                     start=(k == 0), stop=(k == KT - 1))
                nc.vector.tensor_copy(stats[:, t, 0:10], pA[:])
                pS = psp.tile([128, 2], F32, tag="pS", name=f"pS_{t}")
                for k in range(KT):
                    # gate9 cols 8:10 are all-ones summing vectors
                    nc.tensor.matmul(pS[:], sq[:, k, tsl], gate9[:, k, 8:10],
                                     start=(k == 0), stop=(k == KT - 1))
                nc.vector.tensor_copy(stats[:, t, 10:12], pS[:])
            nc.sync.dma_start(stats_o.rearrange("(t p) n -> p t n", p=128),
                              stats[:])

    nc.compile()
    return nc


def _blocks_bf16(n):
    """512-token blocks; small tails rebalanced like the baseline."""
    blocks = [512] * (n // 512)
    r = n % 512
    if r:
        if r < 256 and blocks:
            blocks.pop()
            total = 512 + r
            first = ((total + 1) // 2 + 127) // 128 * 128
            blocks.extend([first, total - first])
        else:
            blocks.append(r)
    return blocks


def _blocks_fp8(n):
    """512-token blocks; rebalance tails <256 (DoubleRow amortizes its
    per-matmul overhead poorly at FD=128)."""
    blocks = [512] * (n // 512)
    r = n % 512
    if r:
        if r < 256 and blocks:
            blocks.pop()
            total = 512 + r
            first = ((total + 1) // 2 + 127) // 128 * 128
            blocks.extend([first, total - first])
        else:
            blocks.append(r)
    return blocks


# ---------------------------------------------------------------- phase 2
def build_phase2(C: int, S: int, act=AF.Gelu):
    """Expert FFN on C slots: CB=C-S bf16 tokens then S fp8 tokens.

    y rows: [0, CB) = bf16 section slots, [CB, C) = fp8 section slots.
    fp8 weights are pre-scaled on host (W1_SCALE/W2_SCALE); the gelu
    activation un-scales mm1 (scale=1/W1_SCALE) and cw8r carries
    cw/W2_SCALE so mm2's scale folds into the existing combine mult."""
    CB = C - S
    bblocks = _blocks_bf16(CB)
    fblocks = _blocks_fp8(S)
    nc = bacc.Bacc("TRN2", target_bir_lowering=False, debug=False,
                   num_devices=N_CORES)
    xbT = nc.dram_tensor("xbT", [D, max(CB, 1)], BF16,
                         kind="ExternalInput").ap()
    xqT = nc.dram_tensor("xqT", [D, max(S, 1)], F8, kind="ExternalInput").ap()
    w1b_d = nc.dram_tensor("w1b", [D, H], BF16, kind="ExternalInput").ap()
    w2b_d = nc.dram_tensor("w2b", [H, D], BF16, kind="ExternalInput").ap()
    w1q_d = nc.dram_tensor("w1q", [D, H], F8, kind="ExternalInput").ap()
    w2q_d = nc.dram_tensor("w2q", [H, D], F8, kind="ExternalInput").ap()
    b1r_d = nc.dram_tensor("b1r", [128, HT], F32, kind="ExternalInput").ap()
    cwbr_d = nc.dram_tensor("cwbr", [128, max(CB // 128, 1)], F32,
                            kind="ExternalInput").ap()
    cw8r_d = nc.dram_tensor("cw8r", [128, max(S // 128, 1)], F32,
                            kind="ExternalInput").ap()
    y_o = nc.dram_tensor("y", [C, D], F32, kind="ExternalOutput").ap()

    xbT_r = xbT.rearrange("(k p) t -> p k t", p=128)
    xqT_r = xqT.rearrange("(k p) t -> p k t", p=128)
    w1b_r = w1b_d.rearrange("(k p) h -> p k h", p=128)
    w2b_r = w2b_d.rearrange("(k p) d -> p k d", p=128)
    w1q_r = w1q_d.rearrange("(k p) h -> p k h", p=128)
    w2q_r = w2q_d.rearrange("(k p) d -> p k d", p=128)

    with tile.TileContext(nc) as tc:
        import contextlib
        with contextlib.ExitStack() as ctx:
            const = ctx.enter_context(tc.tile_pool(name="const", bufs=1))
            wpool = ctx.enter_context(tc.tile_pool(name="w", bufs=2))
            xpool = ctx.enter_context(tc.tile_pool(name="xp", bufs=2))
            hpool = ctx.enter_context(tc.tile_pool(name="h", bufs=34))
            opool = ctx.enter_context(tc.tile_pool(name="o", bufs=2))
            ps1p = ctx.enter_context(
                tc.tile_pool(name="ps1", bufs=4, space="PSUM"))
            ps2p = ctx.enter_context(
                tc.tile_pool(name="ps2", bufs=4, space="PSUM"))

            # ---- DMA preamble: bf16 weights + block-0 activations first;
            # same issue-order trick as the baseline (block-0 x, then w1 in
            # chunks so mm1 can start early, then the rest).
            if CB:
                xb0 = xpool.tile([128, KT, bblocks[0]], BF16, tag="xb",
                                 name="xb_0",
                                 bufs=1 if len(bblocks) == 1 else None)
                nc.sync.dma_start(xb0[:], xbT_r[:, :, 0:bblocks[0]])
            w1b_sb = wpool.tile([128, KT, H], BF16, tag="w", name="w1b_sb")
            w2b_sb = wpool.tile([128, HT, D], BF16, tag="w", name="w2b_sb")
            if CB:
                nc.sync.dma_start(w1b_sb[:, :, 0:H // 16],
                                  w1b_r[:, :, 0:H // 16])
            b1_sb = const.tile([128, HT], F32)
            nc.sync.dma_start(b1_sb[:], b1r_d[:])
            cwb_sb = const.tile([128, max(CB // 128, 1)], F32)
            nc.sync.dma_start(cwb_sb[:], cwbr_d[:])
            cw8_sb = const.tile([128, max(S // 128, 1)], F32)
            nc.sync.dma_start(cw8_sb[:], cw8r_d[:])
            if CB:
                nc.sync.dma_start(w1b_sb[:, :, H // 16:H // 8],
                                  w1b_r[:, :, H // 16:H // 8])
                for q in range(1, 8):
                    qsl = bass.ts(q, H // 8)
                    nc.sync.dma_start(w1b_sb[:, :, qsl], w1b_r[:, :, qsl])
                nc.sync.dma_start(w2b_sb[:, 0:HT // 2, :],
                                  w2b_r[:, 0:HT // 2, :])
                nc.sync.dma_start(w2b_sb[:, HT // 2:HT, :],
                                  w2b_r[:, HT // 2:HT, :])
            # fp8 activations are small (~1.25MB); queue them early so the
            # fp8 section never waits on them.
            fxs = []
            tok0 = 0
            for b, blk in enumerate(fblocks):
                xq = xpool.tile([128, KT, blk], F8, tag="xq", name=f"xq_{b}")
                nc.sync.dma_start(xq[:], xqT_r[:, :, tok0:tok0 + blk])
                fxs.append(xq)
                tok0 += blk

            # ---- bf16 section (baseline structure) ----
            tok0 = 0
            for b, blk in enumerate(bblocks):
                if b == 0:
                    xb = xb0
                else:
                    xb = xpool.tile([128, KT, blk], BF16, tag="xb",
                                    name=f"xb_{b}")
                    nc.sync.dma_start(xb[:], xbT_r[:, :, tok0:tok0 + blk])
                hts = []
                for ht in range(HT):
                    ps = ps1p.tile([128, blk], F32, tag="ps1",
                                   name=f"bps1_{b}_{ht}")
                    for k in range(KT):
                        nc.tensor.matmul(
                            ps[:], w1b_sb[:, k, ht * 128:(ht + 1) * 128],
                            xb[:, k, :], start=(k == 0), stop=(k == KT - 1))
                    htile = hpool.tile([128, blk], BF16, tag="h",
                                       name=f"bht_{b}_{ht}")
                    nc.scalar.activation(htile[:], ps[:], act,
                                         bias=b1_sb[:, ht:ht + 1])
                    hts.append(htile)
                # issue the fp8-weight DMAs right after the LAST bf16 mm1:
                # their SBUF slots (w-tag ring) free exactly then, and all
                # earlier y-outs are already queued ahead of them.
                if b == len(bblocks) - 1:
                    w1q_sb = wpool.tile([128, KT, H], F8, tag="w",
                                        name="w1q_sb")
                    nc.sync.dma_start(w1q_sb[:], w1q_r[:])
                    w2q_sb = wpool.tile([128, HT, D], F8, tag="w",
                                        name="w2q_sb")
                    nc.sync.dma_start(w2q_sb[:], w2q_r[:])
                S_ = blk // 128
                for g in range(0, S_, 2):
                    gs = min(2, S_ - g)
                    ob = opool.tile([128, 2, D], F32, tag="ob",
                                    name=f"bob_{b}_{g}")
                    for j in range(gs):
                        ts_ = g + j
                        tok_sl = bass.ds(ts_ * 128, 128)
                        ps2 = [ps2p.tile([128, 512], F32, tag="ps2",
                                         name=f"bps2_{b}_{ts_}_{i}")
                               for i in range(D // 512)]
                        for kh in range(HT):
                            for dc in range(D // 512):
                                nc.tensor.matmul(
                                    ps2[dc][:], hts[kh][:, tok_sl],
                                    w2b_sb[:, kh, dc * 512:(dc + 1) * 512],
                                    start=(kh == 0), stop=(kh == HT - 1))
                        tok_i = tok0 // 128 + ts_
                        for dc in range(D // 512):
                            nc.vector.tensor_scalar_mul(
                                ob[:, j, dc * 512:(dc + 1) * 512],
                                ps2[dc][:], cwb_sb[:, tok_i:tok_i + 1])
                    nc.sync.dma_start(
                        y_o[tok0 + g * 128:tok0 + (g + gs) * 128, :]
                        .rearrange("(s p) d -> p s d", p=128),
                        ob[:, 0:gs, :])
                tok0 += blk

            if not CB:
                w1q_sb = wpool.tile([128, KT, H], F8, tag="w", name="w1q_sb")
                nc.sync.dma_start(w1q_sb[:], w1q_r[:])
                w2q_sb = wpool.tile([128, HT, D], F8, tag="w", name="w2q_sb")
                nc.sync.dma_start(w2q_sb[:], w2q_r[:])

            # ---- fp8 section: e4m3 DoubleRow matmuls ----
            tok0 = 0
            for b, blk in enumerate(fblocks):
                xq = fxs[b]
                hps = []
                for ht in range(HT):
                    ps = ps1p.tile([128, blk], F32, tag="ps1",
                                   name=f"fps1_{b}_{ht}")
                    for kp in range(KT // 2):
                        nc.tensor.matmul(
                            ps[:],
                            w1q_sb[:, 2 * kp:2 * kp + 2,
                                   ht * 128:(ht + 1) * 128],
                            xq[:, 2 * kp:2 * kp + 2, :],
                            start=(kp == 0), stop=(kp == KT // 2 - 1),
                            perf_mode=DR)
                    if ht % 2 == 0:
                        hp = hpool.tile([128, 2, blk], F8, tag="h",
                                        name=f"fh_{b}_{ht // 2}")
                        hps.append(hp)
                    nc.scalar.activation(hps[-1][:, ht % 2, :], ps[:],
                                         act, bias=b1_sb[:, ht:ht + 1],
                                         scale=1.0 / W1_SCALE)
                S_ = blk // 128
                for g in range(0, S_, 2):
                    gs = min(2, S_ - g)
                    ob = opool.tile([128, 2, D], F32, tag="ob",
                                    name=f"fob_{b}_{g}")
                    for j in range(gs):
                        ts_ = g + j
                        tok_sl = bass.ds(ts_ * 128, 128)
                        ps2 = [ps2p.tile([128, 512], F32, tag="ps2",
                                         name=f"fps2_{b}_{ts_}_{i}")
                               for i in range(D // 512)]
                        for khp in range(HT // 2):
                            for dc in range(D // 512):
                                nc.tensor.matmul(
                                    ps2[dc][:], hps[khp][:, :, tok_sl],
                                    w2q_sb[:, 2 * khp:2 * khp + 2,
                                           dc * 512:(dc + 1) * 512],
                                    start=(khp == 0),
                                    stop=(khp == HT // 2 - 1),
                                    perf_mode=DR)
                        tok_i = tok0 // 128 + ts_
                        for dc in range(D // 512):
                            nc.vector.tensor_scalar_mul(
                                ob[:, j, dc * 512:(dc + 1) * 512],
                                ps2[dc][:], cw8_sb[:, tok_i:tok_i + 1])
                    nc.sync.dma_start(
                        y_o[CB + tok0 + g * 128:CB + tok0 + (g + gs) * 128, :]
                        .rearrange("(s p) d -> p s d", p=128),
                        ob[:, 0:gs, :])
                tok0 += blk

    nc.compile()
    return nc


# ---------------------------------------------------------------- host
def kernel(x, gate_w, w1, b1, w2, b2, gamma, beta):
    x = np.asarray(x, dtype=np.float32)
    gate_w = np.asarray(gate_w, dtype=np.float32)
    w1 = np.asarray(w1, dtype=np.float32)
    b1 = np.asarray(b1, dtype=np.float32)
    w2 = np.asarray(w2, dtype=np.float32)
    b2 = np.asarray(b2, dtype=np.float32)
    gamma = np.asarray(gamma, dtype=np.float32)
    beta = np.asarray(beta, dtype=np.float32)

    xt = np.ascontiguousarray(x.reshape(T, D))

    # ---- phase 1: router/LN statistics on device ----
    if "p1" not in _cache:
        _cache["p1"] = build_phase1()
    nc1 = _cache["p1"]
    geff = gate_w * gamma[:, None]
    geff16 = geff.astype(BFNP).astype(np.float32)
    gate9 = np.concatenate([geff16, np.ones((D, 2), np.float32)],
                           axis=1).astype(BFNP)
    gate9_r = np.ascontiguousarray(
        gate9.reshape(KT, 128, 10).transpose(1, 0, 2))
    in1 = [{"xT": xt[c * TC:(c + 1) * TC].T.astype(BFNP),
            "gate9": gate9_r} for c in range(N_CORES)]
    res1 = run_bass_kernel_spmd(nc1, in1, list(range(N_CORES)))
    LAST_RESULTS["p1"] = res1
    stats = np.concatenate([res1.results[c]["stats"] for c in range(N_CORES)],
                           axis=0)                       # [T, 10]

    # ---- host: LN scalars + exact softmax/top-2 routing ----
    A = stats[:, :E].astype(np.float64)
    s1 = stats[:, E].astype(np.float64)
    s2 = stats[:, E + 2].astype(np.float64)
    mu = s1 / D
    varr = np.maximum(s2 / D - mu * mu, 0.0)
    rstd = 1.0 / np.sqrt(varr + LN_EPS)
    beta_row = beta.astype(np.float64) @ gate_w.astype(np.float64)
    colsum = geff16.sum(0, dtype=np.float64)
    logits = (A * rstd[:, None] - (mu * rstd)[:, None] * colsum[None, :]
              + beta_row[None, :])

    xn = (xt - mu.astype(np.float32)[:, None]) * rstd.astype(np.float32)[:, None]

    # The device logits carry bf16 noise; the top-2 SET is only at risk
    # where the 2nd/3rd logits are close. Recheck those tokens with an
    # exact f64 LayerNorm + router (mirrors the reference arithmetic).
    ls = np.sort(logits, axis=-1)
    flagged = (ls[:, -2] - ls[:, -3]) < 0.08
    if flagged.any():
        xfl = xt[flagged].astype(np.float64)
        muf = xfl.mean(-1, keepdims=True)
        varf = ((xfl - muf) ** 2).mean(-1, keepdims=True)
        xnf = ((xfl - muf) / np.sqrt(varf + LN_EPS)).astype(np.float32)
        logits[flagged] = (xnf.astype(np.float64) @ geff.astype(np.float64)
                           + beta_row[None, :])
        xn[flagged] = xnf

    top2 = np.argsort(-logits, axis=-1, kind="stable")[:, :TOP_K]
    # Renormalized top-2 weights depend only on the two selected logits:
    # compute those two dot products exactly so cw matches the reference.
    g_sel = geff.astype(np.float64).T[top2]                  # [T, 2, D]
    l_sel = (np.einsum("td,tkd->tk", xn.astype(np.float64), g_sel)
             + beta_row[top2])
    wts = 1.0 / (1.0 + np.exp(-(l_sel - l_sel[:, ::-1])))
    cwf = np.zeros((T, E), np.float32)
    np.put_along_axis(cwf, top2, wts.astype(np.float32), axis=-1)

    affine = not (np.all(gamma == 1.0) and np.all(beta == 0.0))
    if affine:
        xn = xn * gamma[None, :] + beta[None, :]

    # ---- host dispatch: per-expert slots sorted ascending by cw ----
    idxs = []
    for e in range(E):
        ix = np.nonzero(cwf[:, e])[0]
        order = np.argsort(cwf[ix, e], kind="stable")
        idxs.append(ix[order])
    counts = [len(ix) for ix in idxs]
    C = max(128, ((max(counts) + 127) // 128) * 128)
    S = int(round(C * FP8_FRAC / 128)) * 128
    S = max(0, min(S, C))
    CB = C - S

    key = ("p2", C, S)
    if key not in _cache:
        _cache[key] = build_phase2(C, S)
    nc2 = _cache[key]

    in2 = []
    f8_reals = []
    bf_reals = []
    for e in range(E):
        ix = idxs[e]
        npad = C - len(ix)
        nf8 = max(0, S - npad)          # real tokens in fp8 slots
        f8_tok = ix[:nf8]
        bf_tok = ix[nf8:]
        f8_reals.append(f8_tok)
        bf_reals.append(bf_tok)

        xq = np.zeros((S, D), dtype=E4NP)
        if len(f8_tok):
            xq[S - len(f8_tok):] = xn[f8_tok].astype(E4NP)
        xb = np.zeros((CB, D), dtype=BFNP)
        if len(bf_tok):
            xb[CB - len(bf_tok):] = xn[bf_tok].astype(BFNP)
        cw8 = np.zeros((max(S, 128),), np.float32)
        if len(f8_tok):
            cw8[S - len(f8_tok):S] = cwf[f8_tok, e] / W2_SCALE
        cwb = np.zeros((max(CB, 128),), np.float32)
        if len(bf_tok):
            cwb[CB - len(bf_tok):CB] = cwf[bf_tok, e]
        in2.append({
            "xbT": np.ascontiguousarray(xb.T),
            "xqT": np.ascontiguousarray(xq.T),
            "w1b": np.ascontiguousarray(w1[e].astype(BFNP)),
            "w2b": np.ascontiguousarray(w2[e].astype(BFNP)),
            "w1q": np.ascontiguousarray((w1[e] * W1_SCALE).astype(E4NP)),
            "w2q": np.ascontiguousarray((w2[e] * W2_SCALE).astype(E4NP)),
            "b1r": np.ascontiguousarray(b1[e].reshape(HT, 128).T),
            "cwbr": np.ascontiguousarray(
                cwb.reshape(-1, 128).T[:, :max(CB // 128, 1)]),
            "cw8r": np.ascontiguousarray(
                cw8.reshape(-1, 128).T[:, :max(S // 128, 1)]),
        })
    res2 = run_bass_kernel_spmd(nc2, in2, list(range(N_CORES)))
    LAST_RESULTS["p2"] = res2

    # ---- host combine: scatter-add + residual (+ per-expert b2) ----
    out = xt.copy()
    b2_any = bool(np.any(b2))
    for e in range(E):
        y = res2.results[e]["y"]            # [C, D]
        f8_tok, bf_tok = f8_reals[e], bf_reals[e]
        if len(bf_tok):
            out[bf_tok] += y[CB - len(bf_tok):CB]
        if len(f8_tok):
            out[f8_tok] += y[C - len(f8_tok):C]
        if b2_any:
            if len(bf_tok):
                out[bf_tok] += cwf[bf_tok, e][:, None] * b2[e][None, :]
            if len(f8_tok):
                out[f8_tok] += cwf[f8_tok, e][:, None] * b2[e][None, :]
    return out.reshape(B, L, D)
